# revision 3
# baseline (speedup 1.0000x reference)
"""LAHGCN hypergraph-conv kernel for 8 Trainium2 NeuronCores (bf16).

Math (per reference):
  smooth(x) = Dv^-1/2 H De^-1 H^T Dv^-1/2 x  (S),  branches k=0..3:
  hidden_k = relu(S(x_k W1_k + 1 b1_k));  out = concat(hidden) W2 + b2;  res = S out.

Key restructuring vs the padded-one-hot baseline:
  * W1 commutes with the edge aggregation: H^T(dv*(x_k W1_k)) = (H^T(dv*x_k)) W1_k,
    so we upload host-prescaled dv*x (bf16, replicated to every core) and gather
    x-rows directly -- no N-side x@W1 pass and no y AllGather.
    ef_k = de * (z_k W1_k + t b1_k) with z = H^T(dv x), t = H^T dv.
  * All four gather passes (edge-sorted x2, node-sorted x2) use
    dma_gather(prepare_only=True) + trigger_dma so the GpSimd engine only pays
    descriptor generation; transfers queue in the SWDGE ring and drain at SDMA
    rate, overlapped with the one-hot TensorE accumulation.
  * No dense H slabs: the second smooth (C_OUT padded 128) is pure gather too,
    reusing the same index/segment streams as the first smooth.
  * Degree scalings: dv folded into the uploaded x; de on the edge passes;
    dv^2 post-W2 (relu commutes with dv>=0); final dv on output;
    b2 via host-side rank-1 s1 = S@1 correction.
"""
import numpy as np
import ml_dtypes

BF16 = ml_dtypes.bfloat16
N, E, NNZ = 50000, 20000, 1600000
CONCAT, C_IN, C_HID = 4, 256, 256
C = CONCAT * C_HID            # 1024
C_OUT, C_OUT_P = 40, 128
W = 8
NPC_R, EPC_R = N // W, E // W           # 6250, 2500 real per core
NBLK, EBLK = 49, 20
NPC, EPC = NBLK * 128, EBLK * 128       # 6272, 2560 padded per core
NP_, EP_ = W * NPC, W * EPC             # 50176, 20480
NHALF = NP_ // 2                        # 25088 (int16 gather index split)
BATCH = 8                               # chunks per dma_gather (1024 idx max)
USE_PREP = False                        # prepare_only + trigger_dma pipelining
NQ = 2                                  # SWDGE queues used for gathers


def _wrap_idx(idx):
    """[L] int -> [128, L/16] int16 wrapped layout, replicated across q7 cores."""
    L = len(idx)
    assert L % 16 == 0
    a = np.full((16, L // 16), 0, np.int16)
    a[np.arange(L) % 16, np.arange(L) // 16] = idx.astype(np.int16)
    return np.tile(a, (8, 1))


def _streams_var(rows, segpos, kbs):
    """Flat index stream + seg table with per-block chunk counts kbs."""
    total = sum(kbs)
    idx = np.zeros(total * 128, np.int64)
    seg = np.full((128, total), -1.0, np.float32)
    off = 0
    for r, p, kb in zip(rows, segpos, kbs):
        n = len(r)
        assert n <= kb * 128
        idx[off * 128:off * 128 + n] = r
        cols = off + np.arange(n) // 128
        seg[np.arange(n) % 128, cols] = p.astype(np.float32)
        off += kb
    return idx, seg.astype(BF16)


def _prep(node_idx, edge_idx, dv_is, de_inv, t_full):
    """Host-side prep: sorted gather streams + per-core scale tables."""
    nrow = (node_idx // NPC_R) * NPC + node_idx % NPC_R    # node -> padded row
    erow = (edge_idx // EPC_R) * EPC + edge_idx % EPC_R    # edge -> padded row
    p1 = np.argsort(edge_idx, kind="stable")
    e1, n1 = edge_idx[p1], nrow[p1]
    p2 = np.argsort(node_idx, kind="stable")
    n2, e2 = node_idx[p2], erow[p2]
    per = []
    for c in range(W):
        m1 = (e1 >= c * EPC_R) & (e1 < (c + 1) * EPC_R)
        el = e1[m1] - c * EPC_R
        nr = n1[m1]
        lo_rows, lo_pos, hi_rows, hi_pos = [], [], [], []
        for b in range(EBLK):
            mb = (el >= b * 128) & (el < (b + 1) * 128)
            rb, pb = nr[mb], el[mb] - b * 128
            lo = rb < NHALF
            lo_rows.append(rb[lo]); lo_pos.append(pb[lo])
            hi_rows.append(rb[~lo] - NHALF); hi_pos.append(pb[~lo])
        m2 = (n2 >= c * NPC_R) & (n2 < (c + 1) * NPC_R)
        nl = n2[m2] - c * NPC_R
        er = e2[m2]
        c_rows, c_pos = [], []
        for b in range(NBLK):
            mb = (nl >= b * 128) & (nl < (b + 1) * 128)
            c_rows.append(er[mb]); c_pos.append(nl[mb] - b * 128)
        per.append((lo_rows, lo_pos, hi_rows, hi_pos, c_rows, c_pos))
    kbA = [max(1, max((len(p[0][b]) + 127) // 128 for p in per)) for b in range(EBLK)]
    kbB = [max(1, max((len(p[2][b]) + 127) // 128 for p in per)) for b in range(EBLK)]
    kbC = [max(1, max((len(p[4][b]) + 127) // 128 for p in per)) for b in range(NBLK)]
    cores = []
    for c in range(W):
        lo_rows, lo_pos, hi_rows, hi_pos, c_rows, c_pos = per[c]
        iA, sA = _streams_var(lo_rows, lo_pos, kbA)
        iB, sB = _streams_var(hi_rows, hi_pos, kbB)
        iC, sC = _streams_var(c_rows, c_pos, kbC)
        dv = np.zeros(NPC, np.float32)
        dv[:NPC_R] = dv_is[c * NPC_R:(c + 1) * NPC_R]
        de = np.zeros(EPC, np.float32)
        de[:EPC_R] = de_inv[c * EPC_R:(c + 1) * EPC_R]
        t = np.zeros(EPC, np.float32)
        t[:EPC_R] = t_full[c * EPC_R:(c + 1) * EPC_R]
        cores.append(dict(
            idxA=_wrap_idx(iA), segA=sA, idxB=_wrap_idx(iB), segB=sB,
            idxC=_wrap_idx(iC), segC=sC,
            t_row=t.reshape(1, EPC).astype(BF16),
            dv_blk=dv.reshape(NBLK, 128).T.copy(),
            dvsq_blk=(dv * dv).reshape(NBLK, 128).T.copy(),
            de_blk=de.reshape(EBLK, 128).T.copy()))
    return cores, tuple(kbA), tuple(kbB), tuple(kbC)


def _build(kbA, kbB, kbC):
    import concourse.bass as bass
    import concourse.mybir as mybir
    from concourse import bacc, masks
    from concourse.tile import TileContext

    f32, bf16, i16 = mybir.dt.float32, mybir.dt.bfloat16, mybir.dt.int16
    sumA, sumB, sumC = sum(kbA), sum(kbB), sum(kbC)
    oA = np.concatenate([[0], np.cumsum(kbA)]).tolist()
    oB = np.concatenate([[0], np.cumsum(kbB)]).tolist()
    oC = np.concatenate([[0], np.cumsum(kbC)]).tolist()

    nc = bacc.Bacc("TRN2", num_devices=W, num_swdge_queues=NQ)
    T = lambda n, s, d=bf16: nc.dram_tensor(n, s, d, kind="ExternalInput")
    dvx = T("dvx", [NP_, C])                 # host-prescaled dv*x, branch-major
    W1 = T("W1", [CONCAT, C_IN, C_HID])
    b1c = T("b1c", [1, C])
    t_row_d = T("t_row", [1, EPC])
    W2p = T("W2p", [C, C_OUT_P])
    dv_blk = T("dv_blk", [128, NBLK], f32); dvsq_blk = T("dvsq_blk", [128, NBLK], f32)
    de_blk = T("de_blk", [128, EBLK], f32)
    idxA = T("idxA", [128, sumA * 8], i16); segA = T("segA", [128, sumA])
    idxB = T("idxB", [128, sumB * 8], i16); segB = T("segB", [128, sumB])
    idxC = T("idxC", [128, sumC * 8], i16); segC = T("segC", [128, sumC])
    iota_d = T("iota", [128, 128])
    out_own = nc.dram_tensor("out_own", [NPC, C_OUT_P], f32, kind="ExternalOutput")
    I = lambda n, s: nc.dram_tensor(n, s, bf16, kind="Internal")
    S = lambda n, s: nc.dram_tensor(n, s, bf16, kind="Internal", addr_space="Shared")
    ef_own, ef_full = I("ef_own", [EPC, C]), S("ef_full", [EP_, C])
    y2_own, y2_full = I("y2_own", [NPC, C_OUT_P]), S("y2_full", [NP_, C_OUT_P])
    ef2_own, ef2_full = I("ef2_own", [EPC, C_OUT_P]), S("ef2_full", [EP_, C_OUT_P])
    RG = [list(range(W))]
    AG = lambda i, o: nc.gpsimd.collective_compute(
        "AllGather", mybir.AluOpType.bypass, replica_groups=RG, ins=[i[:]], outs=[o[:]])

    with TileContext(nc) as tc:
        with tc.tile_pool(name="const", bufs=1) as cp:
            w1_sb = cp.tile([128, CONCAT * 2 * C_HID], bf16)     # f=(k*2+q) -> 256 cols
            for k in range(CONCAT):
                for q in range(2):
                    nc.sync.dma_start(
                        w1_sb[:, (k * 2 + q) * C_HID:(k * 2 + q + 1) * C_HID],
                        W1[k, q * 128:(q + 1) * 128, :])
            w2_sb = cp.tile([128, 8 * C_OUT_P], bf16)
            for f in range(8):
                nc.sync.dma_start(w2_sb[:, f * C_OUT_P:(f + 1) * C_OUT_P],
                                  W2p[f * 128:(f + 1) * 128, :])
            b1_sb = cp.tile([1, C], bf16); nc.sync.dma_start(b1_sb[:], b1c[:])
            t_sb = cp.tile([1, EPC], bf16); nc.sync.dma_start(t_sb[:], t_row_d[:])
            iota_sb = cp.tile([128, 128], bf16); nc.sync.dma_start(iota_sb[:], iota_d[:])
            ident = cp.tile([128, 128], bf16); masks.make_identity(nc, ident[:])
            dv_sb = cp.tile([128, NBLK], f32); nc.sync.dma_start(dv_sb[:], dv_blk[:])
            dvsq_sb = cp.tile([128, NBLK], f32); nc.sync.dma_start(dvsq_sb[:], dvsq_blk[:])
            de_sb = cp.tile([128, EBLK], f32); nc.sync.dma_start(de_sb[:], de_blk[:])
            iA = cp.tile([128, sumA * 8], i16); nc.sync.dma_start(iA[:], idxA[:])
            iB = cp.tile([128, sumB * 8], i16); nc.sync.dma_start(iB[:], idxB[:])
            iC = cp.tile([128, sumC * 8], i16); nc.scalar.dma_start(iC[:], idxC[:])
            sA = cp.tile([128, sumA], bf16); nc.scalar.dma_start(sA[:], segA[:])
            sB = cp.tile([128, sumB], bf16); nc.scalar.dma_start(sB[:], segB[:])
            sC = cp.tile([128, sumC], bf16); nc.scalar.dma_start(sC[:], segC[:])

            mm = lambda *a, **kw: nc.tensor.matmul(*a, skip_group_check=True, **kw)
            qsems = [nc.alloc_semaphore(f"gq{q}") for q in range(NQ)] if USE_PREP else None
            qctr = [0]

            def seg_pass(kb, off, idx_sb, seg_sb, src_ap, elem, pool, ps,
                         start_stream, stop_stream, tag):
                """Gather + one-hot-matmul accumulation for one block's stream."""
                for s in range(0, kb, BATCH):
                    nch = min(BATCH, kb - s)
                    k0 = off + s
                    g = pool.tile([128, BATCH, elem], bf16, tag=tag + "g")
                    if USE_PREP:
                        q = qctr[0] % NQ
                        qctr[0] += 1
                        nc.gpsimd.dma_gather(
                            out_ap=g[:, :nch, :], in_ap=src_ap,
                            idxs_ap=idx_sb[:, k0 * 8:(k0 + nch) * 8],
                            num_idxs=nch * 128, num_idxs_reg=nch * 128,
                            elem_size=elem, prepare_only=True, sem=qsems[q],
                            queue_num=q)
                        nc.gpsimd.trigger_dma(count=None, queue_num=q)
                    else:
                        nc.gpsimd.dma_gather(
                            out_ap=g[:, :nch, :], in_ap=src_ap,
                            idxs_ap=idx_sb[:, k0 * 8:(k0 + nch) * 8],
                            num_idxs=nch * 128, num_idxs_reg=nch * 128,
                            elem_size=elem)
                    oh = pool.tile([128, BATCH, 128], bf16, tag=tag + "o")
                    nc.vector.tensor_tensor(
                        out=oh[:, :nch, :],
                        in0=iota_sb[:, None, :].broadcast_to([128, nch, 128]),
                        in1=seg_sb[:, k0:k0 + nch, None].broadcast_to([128, nch, 128]),
                        op=mybir.AluOpType.is_equal)
                    for j in range(nch):
                        first = start_stream and (s == 0 and j == 0)
                        last = stop_stream and (s + j == kb - 1)
                        for h in range((elem + 511) // 512):
                            w_ = min(512, elem - h * 512)
                            mm(ps[:, h * 512:h * 512 + w_],
                               lhsT=oh[:, j, :], rhs=g[:, j, h * 512:h * 512 + w_],
                               start=first, stop=last)

            # ---- phase B': z = H^T(dv x); ef = de * (z_k W1_k + t b1_k) ----
            with tc.tile_pool(name="pb", bufs=3) as pb, \
                 tc.tile_pool(name="pbz", bufs=2, space="PSUM") as pbz, \
                 tc.tile_pool(name="pbt", bufs=1, space="PSUM") as pbt, \
                 tc.tile_pool(name="pbe", bufs=1, space="PSUM") as pbe:
                for b in range(EBLK):
                    pz = pbz.tile([128, C], f32, tag="pz")
                    seg_pass(kbA[b], oA[b], iA, sA, dvx[0:NHALF, :], C,
                             pb, pz, True, False, "A")
                    seg_pass(kbB[b], oB[b], iB, sB, dvx[NHALF:NP_, :], C,
                             pb, pz, False, True, "B")
                    z_sb = pb.tile([128, C], bf16, tag="zsb")
                    nc.vector.tensor_copy(z_sb[:], pz[:])
                    pt = pbt.tile([128, C], bf16, tag="pt")
                    for f in range(8):
                        nc.tensor.transpose(pt[:, f * 128:(f + 1) * 128],
                                            z_sb[:, f * 128:(f + 1) * 128], ident[:])
                    zt_sb = pb.tile([128, C], bf16, tag="ztsb")
                    nc.vector.tensor_copy(zt_sb[:], pt[:])
                    pef = pbe.tile([128, C], f32, tag="pef")
                    tb = t_sb[:, b * 128:(b + 1) * 128]
                    mm(pef[:, :512], lhsT=tb, rhs=b1_sb[:, :512], start=True, stop=False)
                    mm(pef[:, 512:], lhsT=tb, rhs=b1_sb[:, 512:], start=True, stop=False)
                    for k in range(CONCAT):
                        for qh in range(2):
                            f = 2 * k + qh
                            mm(pef[:, k * C_HID:(k + 1) * C_HID],
                               lhsT=zt_sb[:, f * 128:(f + 1) * 128],
                               rhs=w1_sb[:, f * C_HID:(f + 1) * C_HID],
                               start=False, stop=(qh == 1))
                    ef_sb = pb.tile([128, C], bf16, tag="efsb")
                    nc.vector.tensor_tensor(
                        out=ef_sb[:], in0=pef[:],
                        in1=de_sb[:, b:b + 1].broadcast_to([128, C]),
                        op=mybir.AluOpType.mult)
                    nc.sync.dma_start(ef_own[b * 128:(b + 1) * 128, :], ef_sb[:])
            AG(ef_own, ef_full)

            # ---- phase C: u = relu(H ef); y2 = dv^2 * (u @ W2) ----
            with tc.tile_pool(name="pc", bufs=3) as pc, \
                 tc.tile_pool(name="pcz", bufs=2, space="PSUM") as pcz, \
                 tc.tile_pool(name="pct", bufs=1, space="PSUM") as pct, \
                 tc.tile_pool(name="pcy", bufs=2, space="PSUM") as pcy:
                for b in range(NBLK):
                    pu = pcz.tile([128, C], f32, tag="pu")
                    seg_pass(kbC[b], oC[b], iC, sC, ef_full[:], C, pc, pu,
                             True, True, "C")
                    u_sb = pc.tile([128, C], bf16, tag="usb")
                    nc.scalar.activation(out=u_sb[:], in_=pu[:],
                                         func=mybir.ActivationFunctionType.Relu)
                    pt = pct.tile([128, C], bf16, tag="ptc")
                    for f in range(8):
                        nc.tensor.transpose(pt[:, f * 128:(f + 1) * 128],
                                            u_sb[:, f * 128:(f + 1) * 128], ident[:])
                    ut_sb = pc.tile([128, C], bf16, tag="utsb")
                    nc.vector.tensor_copy(ut_sb[:], pt[:])
                    py2 = pcy.tile([128, C_OUT_P], f32, tag="py2")
                    for f in range(8):
                        mm(py2[:], lhsT=ut_sb[:, f * 128:(f + 1) * 128],
                           rhs=w2_sb[:, f * C_OUT_P:(f + 1) * C_OUT_P],
                           start=(f == 0), stop=(f == 7))
                    y2_sb = pc.tile([128, C_OUT_P], bf16, tag="y2sb")
                    nc.vector.tensor_tensor(
                        out=y2_sb[:], in0=py2[:],
                        in1=dvsq_sb[:, b:b + 1].broadcast_to([128, C_OUT_P]),
                        op=mybir.AluOpType.mult)
                    nc.sync.dma_start(y2_own[b * 128:(b + 1) * 128, :], y2_sb[:])
            AG(y2_own, y2_full)

            # ---- phase D: ef2 = de * (H^T y2), same streams as B' ----
            with tc.tile_pool(name="pd", bufs=3) as pd, \
                 tc.tile_pool(name="pdp", bufs=3, space="PSUM") as pdp:
                for b in range(EBLK):
                    ps2 = pdp.tile([128, C_OUT_P], f32, tag="ps2")
                    seg_pass(kbA[b], oA[b], iA, sA, y2_full[0:NHALF, :],
                             C_OUT_P, pd, ps2, True, False, "D")
                    seg_pass(kbB[b], oB[b], iB, sB, y2_full[NHALF:NP_, :],
                             C_OUT_P, pd, ps2, False, True, "E")
                    e2_sb = pd.tile([128, C_OUT_P], bf16, tag="e2sb")
                    nc.vector.tensor_tensor(
                        out=e2_sb[:], in0=ps2[:],
                        in1=de_sb[:, b:b + 1].broadcast_to([128, C_OUT_P]),
                        op=mybir.AluOpType.mult)
                    nc.sync.dma_start(ef2_own[b * 128:(b + 1) * 128, :], e2_sb[:])
            AG(ef2_own, ef2_full)

            # ---- phase E: res = dv * (H ef2), same stream as C ----
            with tc.tile_pool(name="pe", bufs=3) as pe_, \
                 tc.tile_pool(name="pep", bufs=3, space="PSUM") as pep:
                for b in range(NBLK):
                    po = pep.tile([128, C_OUT_P], f32, tag="po")
                    seg_pass(kbC[b], oC[b], iC, sC, ef2_full[:], C_OUT_P,
                             pe_, po, True, True, "F")
                    o_sb = pe_.tile([128, C_OUT_P], f32, tag="osb")
                    nc.vector.tensor_tensor(
                        out=o_sb[:], in0=po[:],
                        in1=dv_sb[:, b:b + 1].broadcast_to([128, C_OUT_P]),
                        op=mybir.AluOpType.mult)
                    nc.sync.dma_start(out_own[b * 128:(b + 1) * 128, :], o_sb[:])
    nc.finalize()
    return nc


_CACHE = {}


def kernel(x_list, W1, b1, W2, b2, node_idx, edge_idx, n_edges, _trace=False,
           _tmpdir=None):
    from concourse import bass_utils
    x_list = np.asarray(x_list, np.float32); W1 = np.asarray(W1, np.float32)
    b1 = np.asarray(b1, np.float32); W2 = np.asarray(W2, np.float32)
    b2 = np.asarray(b2, np.float32)
    node_idx = np.asarray(node_idx, np.int32); edge_idx = np.asarray(edge_idx, np.int32)

    dv = np.bincount(node_idx, minlength=N).astype(np.float32)
    de = np.bincount(edge_idx, minlength=E).astype(np.float32)
    dv_is = np.where(dv > 0, 1.0 / np.sqrt(np.maximum(dv, 1.0)), 0.0).astype(np.float32)
    de_inv = np.where(de > 0, 1.0 / np.maximum(de, 1.0), 0.0).astype(np.float32)
    # t = H^T dv (edge sums of dv) for the b1 rank-1 term
    t_full = np.bincount(edge_idx, weights=dv_is[node_idx], minlength=E).astype(np.float32)
    # s1 = S @ 1 for the host-side b2 rank-1 term
    ef_t = t_full * de_inv
    s1 = dv_is * np.bincount(node_idx, weights=ef_t[edge_idx], minlength=N)

    cores, kbA, kbB, kbC = _prep(node_idx, edge_idx, dv_is, de_inv, t_full)
    key = (kbA, kbB, kbC)
    if key not in _CACHE:
        _CACHE[key] = _build(kbA, kbB, kbC)
    nc = _CACHE[key]

    W2p = np.zeros((C, C_OUT_P), np.float32)
    W2p[:, :C_OUT] = W2
    iota_np = np.tile(np.arange(128, dtype=np.float32), (128, 1))
    # dvx[padded row, (k,cin)] = dv[n] * x[k, n, cin], branch-major columns
    dvx_real = (x_list.transpose(1, 0, 2).reshape(N, C)
                * dv_is[:, None]).astype(BF16)
    dvx_p = np.zeros((NP_, C), BF16)
    for c in range(W):
        dvx_p[c * NPC:c * NPC + NPC_R] = dvx_real[c * NPC_R:(c + 1) * NPC_R]
    in_maps = []
    for c in range(W):
        m = dict(dvx=dvx_p, W1=W1.astype(BF16),
                 b1c=b1.reshape(1, C).astype(BF16), W2p=W2p.astype(BF16),
                 iota=iota_np.astype(BF16), **cores[c])
        in_maps.append(m)
    try:
        res = bass_utils.run_bass_kernel_spmd(nc, in_maps, core_ids=list(range(W)),
                                              trace=_trace, tmpdir=_tmpdir)
    except ModuleNotFoundError:
        res = bass_utils.run_bass_kernel_spmd(nc, in_maps, core_ids=list(range(W)),
                                              trace=False)
    out = np.empty((N, C_OUT), np.float32)
    for c in range(W):
        out[c * NPC_R:(c + 1) * NPC_R] = res.results[c]["out_own"][:NPC_R, :C_OUT]
    out += np.outer(s1, b2)
    kernel._last = res
    return out


# revision 9
# speedup vs baseline: 1.3262x; 1.3262x over previous
"""LAHGCN hypergraph-conv kernel for 8 Trainium2 NeuronCores (bf16).

Math (per reference):
  smooth(x) = Dv^-1/2 H De^-1 H^T Dv^-1/2 x  (S),  branches k=0..3:
  hidden_k = relu(S(x_k W1_k + 1 b1_k));  out = concat(hidden) W2 + b2;  res = S out.

Key restructuring vs the padded-one-hot baseline:
  * W1 commutes with the edge aggregation: H^T(dv*(x_k W1_k)) = (H^T(dv*x_k)) W1_k,
    so we upload host-prescaled dv*x (bf16, replicated to every core) and gather
    x-rows directly -- no N-side x@W1 pass and no y AllGather.
    ef_k = de * (z_k W1_k + t b1_k) with z = H^T(dv x), t = H^T dv.
  * All four gather passes (edge-sorted x2, node-sorted x2) use
    dma_gather(prepare_only=True) + trigger_dma so the GpSimd engine only pays
    descriptor generation; transfers queue in the SWDGE ring and drain at SDMA
    rate, overlapped with the one-hot TensorE accumulation.
  * No dense H slabs: the second smooth (C_OUT padded 128) is pure gather too,
    reusing the same index/segment streams as the first smooth.
  * Degree scalings: dv folded into the uploaded x; de on the edge passes;
    dv^2 post-W2 (relu commutes with dv>=0); final dv on output;
    b2 via host-side rank-1 s1 = S@1 correction.
"""
import numpy as np
import ml_dtypes

BF16 = ml_dtypes.bfloat16
N, E, NNZ = 50000, 20000, 1600000
CONCAT, C_IN, C_HID = 4, 256, 256
C = CONCAT * C_HID            # 1024
C_OUT, C_OUT_P = 40, 128
W = 8
NPC_R, EPC_R = N // W, E // W           # 6250, 2500 real per core
NBLK, EBLK = 49, 20
NPC, EPC = NBLK * 128, EBLK * 128       # 6272, 2560 padded per core
NP_, EP_ = W * NPC, W * EPC             # 50176, 20480
NHALF = NP_ // 2                        # 25088 (int16 gather index split)
BATCH = 8                               # chunks per dma_gather (1024 idx max)
USE_PREP = False                        # prepare_only + trigger_dma pipelining
NQ = 3                                  # SWDGE queues used for gathers (ring
                                        # per queue -> overlapped transfers)


def _wrap_idx(idx):
    """[L] int -> [128, L/16] int16 wrapped layout, replicated across q7 cores."""
    L = len(idx)
    assert L % 16 == 0
    a = np.full((16, L // 16), 0, np.int16)
    a[np.arange(L) % 16, np.arange(L) // 16] = idx.astype(np.int16)
    return np.tile(a, (8, 1))


def _streams_var(rows, segpos, kbs):
    """Flat index stream + seg table with per-block chunk counts kbs."""
    total = sum(kbs)
    idx = np.zeros(total * 128, np.int64)
    seg = np.full((128, total), -1.0, np.float32)
    off = 0
    for r, p, kb in zip(rows, segpos, kbs):
        n = len(r)
        assert n <= kb * 128
        idx[off * 128:off * 128 + n] = r
        cols = off + np.arange(n) // 128
        seg[np.arange(n) % 128, cols] = p.astype(np.float32)
        off += kb
    return idx, seg.astype(BF16)


def _prep(node_idx, edge_idx, dv_is, de_inv, t_full):
    """Host-side prep: sorted gather streams + per-core scale tables."""
    nrow = (node_idx // NPC_R) * NPC + node_idx % NPC_R    # node -> padded row
    erow = (edge_idx // EPC_R) * EPC + edge_idx % EPC_R    # edge -> padded row
    p1 = np.argsort(edge_idx, kind="stable")
    e1, n1 = edge_idx[p1], nrow[p1]
    p2 = np.argsort(node_idx, kind="stable")
    n2, e2 = node_idx[p2], erow[p2]
    per = []
    for c in range(W):
        m1 = (e1 >= c * EPC_R) & (e1 < (c + 1) * EPC_R)
        el = e1[m1] - c * EPC_R
        nr = n1[m1]
        lo_rows, lo_pos, hi_rows, hi_pos = [], [], [], []
        for b in range(EBLK):
            mb = (el >= b * 128) & (el < (b + 1) * 128)
            rb, pb = nr[mb], el[mb] - b * 128
            lo = rb < NHALF
            lo_rows.append(rb[lo]); lo_pos.append(pb[lo])
            hi_rows.append(rb[~lo] - NHALF); hi_pos.append(pb[~lo])
        m2 = (n2 >= c * NPC_R) & (n2 < (c + 1) * NPC_R)
        nl = n2[m2] - c * NPC_R
        er = e2[m2]
        c_rows, c_pos = [], []
        for b in range(NBLK):
            mb = (nl >= b * 128) & (nl < (b + 1) * 128)
            c_rows.append(er[mb]); c_pos.append(nl[mb] - b * 128)
        per.append((lo_rows, lo_pos, hi_rows, hi_pos, c_rows, c_pos))
    kbA = [max(1, max((len(p[0][b]) + 127) // 128 for p in per)) for b in range(EBLK)]
    kbB = [max(1, max((len(p[2][b]) + 127) // 128 for p in per)) for b in range(EBLK)]
    kbC = [max(1, max((len(p[4][b]) + 127) // 128 for p in per)) for b in range(NBLK)]
    cores = []
    for c in range(W):
        lo_rows, lo_pos, hi_rows, hi_pos, c_rows, c_pos = per[c]
        iA, sA = _streams_var(lo_rows, lo_pos, kbA)
        iB, sB = _streams_var(hi_rows, hi_pos, kbB)
        iC, sC = _streams_var(c_rows, c_pos, kbC)
        dv = np.zeros(NPC, np.float32)
        dv[:NPC_R] = dv_is[c * NPC_R:(c + 1) * NPC_R]
        de = np.zeros(EPC, np.float32)
        de[:EPC_R] = de_inv[c * EPC_R:(c + 1) * EPC_R]
        t = np.zeros(EPC, np.float32)
        t[:EPC_R] = t_full[c * EPC_R:(c + 1) * EPC_R]
        cores.append(dict(
            idxA=_wrap_idx(iA), segA=sA, idxB=_wrap_idx(iB), segB=sB,
            idxC=_wrap_idx(iC), segC=sC,
            t_row=t.reshape(1, EPC).astype(BF16),
            dv_blk=dv.reshape(NBLK, 128).T.copy(),
            dvsq_blk=(dv * dv).reshape(NBLK, 128).T.copy(),
            de_blk=de.reshape(EBLK, 128).T.copy()))
    return cores, tuple(kbA), tuple(kbB), tuple(kbC)


def _build(kbA, kbB, kbC):
    import concourse.bass as bass
    import concourse.mybir as mybir
    from concourse import bacc, masks
    from concourse.tile import TileContext

    f32, bf16, i16 = mybir.dt.float32, mybir.dt.bfloat16, mybir.dt.int16
    sumA, sumB, sumC = sum(kbA), sum(kbB), sum(kbC)
    oA = np.concatenate([[0], np.cumsum(kbA)]).tolist()
    oB = np.concatenate([[0], np.cumsum(kbB)]).tolist()
    oC = np.concatenate([[0], np.cumsum(kbC)]).tolist()

    nc = bacc.Bacc("TRN2", num_devices=W, num_swdge_queues=NQ)
    T = lambda n, s, d=bf16: nc.dram_tensor(n, s, d, kind="ExternalInput")
    dvx = T("dvx", [NP_, C])                 # host-prescaled dv*x, branch-major
    W1 = T("W1", [CONCAT, C_IN, C_HID])
    b1c = T("b1c", [1, C])
    t_row_d = T("t_row", [1, EPC])
    W2p = T("W2p", [C, C_OUT_P])
    dv_blk = T("dv_blk", [128, NBLK], f32); dvsq_blk = T("dvsq_blk", [128, NBLK], f32)
    de_blk = T("de_blk", [128, EBLK], f32)
    idxA = T("idxA", [128, sumA * 8], i16); segA = T("segA", [128, sumA])
    idxB = T("idxB", [128, sumB * 8], i16); segB = T("segB", [128, sumB])
    idxC = T("idxC", [128, sumC * 8], i16); segC = T("segC", [128, sumC])
    iota_d = T("iota", [128, 128])
    out_own = nc.dram_tensor("out_own", [NPC, C_OUT_P], f32, kind="ExternalOutput")
    I = lambda n, s: nc.dram_tensor(n, s, bf16, kind="Internal")
    S = lambda n, s: nc.dram_tensor(n, s, bf16, kind="Internal", addr_space="Shared")
    ef_own, ef_full = I("ef_own", [EPC, C]), S("ef_full", [EP_, C])
    y2_own, y2_full = I("y2_own", [NPC, C_OUT_P]), S("y2_full", [NP_, C_OUT_P])
    ef2_own, ef2_full = I("ef2_own", [EPC, C_OUT_P]), S("ef2_full", [EP_, C_OUT_P])
    RG = [list(range(W))]
    AG = lambda i, o: nc.gpsimd.collective_compute(
        "AllGather", mybir.AluOpType.bypass, replica_groups=RG, ins=[i[:]], outs=[o[:]])

    with TileContext(nc) as tc:
        with tc.tile_pool(name="const", bufs=1) as cp:
            w1_sb = cp.tile([128, CONCAT * 2 * C_HID], bf16)     # f=(k*2+q) -> 256 cols
            for k in range(CONCAT):
                for q in range(2):
                    nc.sync.dma_start(
                        w1_sb[:, (k * 2 + q) * C_HID:(k * 2 + q + 1) * C_HID],
                        W1[k, q * 128:(q + 1) * 128, :])
            w2_sb = cp.tile([128, 8 * C_OUT_P], bf16)
            for f in range(8):
                nc.sync.dma_start(w2_sb[:, f * C_OUT_P:(f + 1) * C_OUT_P],
                                  W2p[f * 128:(f + 1) * 128, :])
            b1_sb = cp.tile([1, C], bf16); nc.sync.dma_start(b1_sb[:], b1c[:])
            t_sb = cp.tile([1, EPC], bf16); nc.sync.dma_start(t_sb[:], t_row_d[:])
            iota_sb = cp.tile([128, 128], bf16); nc.sync.dma_start(iota_sb[:], iota_d[:])
            ident = cp.tile([128, 128], bf16); masks.make_identity(nc, ident[:])
            dv_sb = cp.tile([128, NBLK], f32); nc.sync.dma_start(dv_sb[:], dv_blk[:])
            dvsq_sb = cp.tile([128, NBLK], f32); nc.sync.dma_start(dvsq_sb[:], dvsq_blk[:])
            de_sb = cp.tile([128, EBLK], f32); nc.sync.dma_start(de_sb[:], de_blk[:])
            iA = cp.tile([128, sumA * 8], i16); nc.sync.dma_start(iA[:], idxA[:])
            iB = cp.tile([128, sumB * 8], i16); nc.sync.dma_start(iB[:], idxB[:])
            iC = cp.tile([128, sumC * 8], i16); nc.scalar.dma_start(iC[:], idxC[:])
            sA = cp.tile([128, sumA], bf16); nc.scalar.dma_start(sA[:], segA[:])
            sB = cp.tile([128, sumB], bf16); nc.scalar.dma_start(sB[:], segB[:])
            sC = cp.tile([128, sumC], bf16); nc.scalar.dma_start(sC[:], segC[:])

            mm = lambda *a, **kw: nc.tensor.matmul(*a, skip_group_check=True, **kw)
            qsems = [nc.alloc_semaphore(f"gq{q}") for q in range(NQ)] if USE_PREP else None
            qctr = [0]
            qthr = [0] * NQ
            if USE_PREP:
                for q in range(NQ):
                    nc.gpsimd.sem_clear(qsems[q])

            def seg_pass(kb, off, idx_sb, seg_sb, src_ap, elem, pool, ps,
                         start_stream, stop_stream, tag):
                """Gather + one-hot-matmul accumulation for one block's stream."""
                for s in range(0, kb, BATCH):
                    nch = min(BATCH, kb - s)
                    k0 = off + s
                    g = pool.tile([128, BATCH, elem], bf16, tag=tag + "g")
                    gate = None
                    if USE_PREP:
                        q = qctr[0] % NQ
                        qctr[0] += 1
                        nc.gpsimd.dma_gather(
                            out_ap=g[:, :nch, :], in_ap=src_ap,
                            idxs_ap=idx_sb[:, k0 * 8:(k0 + nch) * 8],
                            num_idxs=nch * 128, num_idxs_reg=nch * 128,
                            elem_size=elem, prepare_only=True, sem=qsems[q],
                            queue_num=q)
                        nc.gpsimd.trigger_dma(count=None, queue_num=q)
                        qthr[q] += 16
                        gate = (qsems[q], qthr[q])
                    else:
                        q = qctr[0] % NQ
                        qctr[0] += 1
                        nc.gpsimd.dma_gather(
                            out_ap=g[:, :nch, :], in_ap=src_ap,
                            idxs_ap=idx_sb[:, k0 * 8:(k0 + nch) * 8],
                            num_idxs=nch * 128, num_idxs_reg=nch * 128,
                            elem_size=elem, queue_num=q)
                    oh = pool.tile([128, BATCH, 128], bf16, tag=tag + "o")
                    nc.vector.tensor_tensor(
                        out=oh[:, :nch, :],
                        in0=iota_sb[:, None, :].broadcast_to([128, nch, 128]),
                        in1=seg_sb[:, k0:k0 + nch, None].broadcast_to([128, nch, 128]),
                        op=mybir.AluOpType.is_equal)
                    if gate is not None:
                        nc.tensor.wait_ge(gate[0], gate[1])
                    for j in range(nch):
                        first = start_stream and (s == 0 and j == 0)
                        last = stop_stream and (s + j == kb - 1)
                        for h in range((elem + 511) // 512):
                            w_ = min(512, elem - h * 512)
                            mm(ps[:, h * 512:h * 512 + w_],
                               lhsT=oh[:, j, :], rhs=g[:, j, h * 512:h * 512 + w_],
                               start=first, stop=last)

            # ---- phase B': z = H^T(dv x); ef = de * (z_k W1_k + t b1_k) ----
            with tc.tile_pool(name="pb", bufs=3) as pb, \
                 tc.tile_pool(name="pbz", bufs=2, space="PSUM") as pbz, \
                 tc.tile_pool(name="pbt", bufs=1, space="PSUM") as pbt, \
                 tc.tile_pool(name="pbe", bufs=1, space="PSUM") as pbe:
                for b in range(EBLK):
                    pz = pbz.tile([128, C], f32, tag="pz")
                    seg_pass(kbA[b], oA[b], iA, sA, dvx[0:NHALF, :], C,
                             pb, pz, True, False, "A")
                    seg_pass(kbB[b], oB[b], iB, sB, dvx[NHALF:NP_, :], C,
                             pb, pz, False, True, "B")
                    z_sb = pb.tile([128, C], bf16, tag="zsb")
                    nc.vector.tensor_copy(z_sb[:], pz[:])
                    pt = pbt.tile([128, C], bf16, tag="pt")
                    for f in range(8):
                        nc.tensor.transpose(pt[:, f * 128:(f + 1) * 128],
                                            z_sb[:, f * 128:(f + 1) * 128], ident[:])
                    zt_sb = pb.tile([128, C], bf16, tag="ztsb")
                    nc.vector.tensor_copy(zt_sb[:], pt[:])
                    pef = pbe.tile([128, C], f32, tag="pef")
                    tb = t_sb[:, b * 128:(b + 1) * 128]
                    mm(pef[:, :512], lhsT=tb, rhs=b1_sb[:, :512], start=True, stop=False)
                    mm(pef[:, 512:], lhsT=tb, rhs=b1_sb[:, 512:], start=True, stop=False)
                    for k in range(CONCAT):
                        for qh in range(2):
                            f = 2 * k + qh
                            mm(pef[:, k * C_HID:(k + 1) * C_HID],
                               lhsT=zt_sb[:, f * 128:(f + 1) * 128],
                               rhs=w1_sb[:, f * C_HID:(f + 1) * C_HID],
                               start=False, stop=(qh == 1))
                    ef_sb = pb.tile([128, C], bf16, tag="efsb")
                    nc.vector.tensor_tensor(
                        out=ef_sb[:], in0=pef[:],
                        in1=de_sb[:, b:b + 1].broadcast_to([128, C]),
                        op=mybir.AluOpType.mult)
                    nc.sync.dma_start(ef_own[b * 128:(b + 1) * 128, :], ef_sb[:])
            AG(ef_own, ef_full)

            # ---- phase C: u = relu(H ef); y2 = dv^2 * (u @ W2) ----
            with tc.tile_pool(name="pc", bufs=3) as pc, \
                 tc.tile_pool(name="pcz", bufs=2, space="PSUM") as pcz, \
                 tc.tile_pool(name="pct", bufs=1, space="PSUM") as pct, \
                 tc.tile_pool(name="pcy", bufs=2, space="PSUM") as pcy:
                for b in range(NBLK):
                    pu = pcz.tile([128, C], f32, tag="pu")
                    seg_pass(kbC[b], oC[b], iC, sC, ef_full[:], C, pc, pu,
                             True, True, "C")
                    u_sb = pc.tile([128, C], bf16, tag="usb")
                    nc.scalar.activation(out=u_sb[:], in_=pu[:],
                                         func=mybir.ActivationFunctionType.Relu)
                    pt = pct.tile([128, C], bf16, tag="ptc")
                    for f in range(8):
                        nc.tensor.transpose(pt[:, f * 128:(f + 1) * 128],
                                            u_sb[:, f * 128:(f + 1) * 128], ident[:])
                    ut_sb = pc.tile([128, C], bf16, tag="utsb")
                    nc.vector.tensor_copy(ut_sb[:], pt[:])
                    py2 = pcy.tile([128, C_OUT_P], f32, tag="py2")
                    for f in range(8):
                        mm(py2[:], lhsT=ut_sb[:, f * 128:(f + 1) * 128],
                           rhs=w2_sb[:, f * C_OUT_P:(f + 1) * C_OUT_P],
                           start=(f == 0), stop=(f == 7))
                    y2_sb = pc.tile([128, C_OUT_P], bf16, tag="y2sb")
                    nc.vector.tensor_tensor(
                        out=y2_sb[:], in0=py2[:],
                        in1=dvsq_sb[:, b:b + 1].broadcast_to([128, C_OUT_P]),
                        op=mybir.AluOpType.mult)
                    nc.sync.dma_start(y2_own[b * 128:(b + 1) * 128, :], y2_sb[:])
            AG(y2_own, y2_full)

            # ---- phase D: ef2 = de * (H^T y2), same streams as B' ----
            with tc.tile_pool(name="pd", bufs=3) as pd, \
                 tc.tile_pool(name="pdp", bufs=3, space="PSUM") as pdp:
                for b in range(EBLK):
                    ps2 = pdp.tile([128, C_OUT_P], f32, tag="ps2")
                    seg_pass(kbA[b], oA[b], iA, sA, y2_full[0:NHALF, :],
                             C_OUT_P, pd, ps2, True, False, "D")
                    seg_pass(kbB[b], oB[b], iB, sB, y2_full[NHALF:NP_, :],
                             C_OUT_P, pd, ps2, False, True, "E")
                    e2_sb = pd.tile([128, C_OUT_P], bf16, tag="e2sb")
                    nc.vector.tensor_tensor(
                        out=e2_sb[:], in0=ps2[:],
                        in1=de_sb[:, b:b + 1].broadcast_to([128, C_OUT_P]),
                        op=mybir.AluOpType.mult)
                    nc.sync.dma_start(ef2_own[b * 128:(b + 1) * 128, :], e2_sb[:])
            AG(ef2_own, ef2_full)

            # ---- phase E: res = dv * (H ef2), same stream as C ----
            with tc.tile_pool(name="pe", bufs=3) as pe_, \
                 tc.tile_pool(name="pep", bufs=3, space="PSUM") as pep:
                for b in range(NBLK):
                    po = pep.tile([128, C_OUT_P], f32, tag="po")
                    seg_pass(kbC[b], oC[b], iC, sC, ef2_full[:], C_OUT_P,
                             pe_, po, True, True, "F")
                    o_sb = pe_.tile([128, C_OUT_P], f32, tag="osb")
                    nc.vector.tensor_tensor(
                        out=o_sb[:], in0=po[:],
                        in1=dv_sb[:, b:b + 1].broadcast_to([128, C_OUT_P]),
                        op=mybir.AluOpType.mult)
                    nc.sync.dma_start(out_own[b * 128:(b + 1) * 128, :], o_sb[:])
    nc.finalize()
    return nc


_CACHE = {}


def kernel(x_list, W1, b1, W2, b2, node_idx, edge_idx, n_edges, _trace=False,
           _tmpdir=None):
    from concourse import bass_utils
    x_list = np.asarray(x_list, np.float32); W1 = np.asarray(W1, np.float32)
    b1 = np.asarray(b1, np.float32); W2 = np.asarray(W2, np.float32)
    b2 = np.asarray(b2, np.float32)
    node_idx = np.asarray(node_idx, np.int32); edge_idx = np.asarray(edge_idx, np.int32)

    dv = np.bincount(node_idx, minlength=N).astype(np.float32)
    de = np.bincount(edge_idx, minlength=E).astype(np.float32)
    dv_is = np.where(dv > 0, 1.0 / np.sqrt(np.maximum(dv, 1.0)), 0.0).astype(np.float32)
    de_inv = np.where(de > 0, 1.0 / np.maximum(de, 1.0), 0.0).astype(np.float32)
    # t = H^T dv (edge sums of dv) for the b1 rank-1 term
    t_full = np.bincount(edge_idx, weights=dv_is[node_idx], minlength=E).astype(np.float32)
    # s1 = S @ 1 for the host-side b2 rank-1 term
    ef_t = t_full * de_inv
    s1 = dv_is * np.bincount(node_idx, weights=ef_t[edge_idx], minlength=N)

    cores, kbA, kbB, kbC = _prep(node_idx, edge_idx, dv_is, de_inv, t_full)
    key = (kbA, kbB, kbC)
    if key not in _CACHE:
        _CACHE[key] = _build(kbA, kbB, kbC)
    nc = _CACHE[key]

    W2p = np.zeros((C, C_OUT_P), np.float32)
    W2p[:, :C_OUT] = W2
    iota_np = np.tile(np.arange(128, dtype=np.float32), (128, 1))
    # dvx[padded row, (k,cin)] = dv[n] * x[k, n, cin], branch-major columns
    dvx_real = (x_list.transpose(1, 0, 2).reshape(N, C)
                * dv_is[:, None]).astype(BF16)
    dvx_p = np.zeros((NP_, C), BF16)
    for c in range(W):
        dvx_p[c * NPC:c * NPC + NPC_R] = dvx_real[c * NPC_R:(c + 1) * NPC_R]
    in_maps = []
    for c in range(W):
        m = dict(dvx=dvx_p, W1=W1.astype(BF16),
                 b1c=b1.reshape(1, C).astype(BF16), W2p=W2p.astype(BF16),
                 iota=iota_np.astype(BF16), **cores[c])
        in_maps.append(m)
    try:
        res = bass_utils.run_bass_kernel_spmd(nc, in_maps, core_ids=list(range(W)),
                                              trace=_trace, tmpdir=_tmpdir)
    except ModuleNotFoundError:
        res = bass_utils.run_bass_kernel_spmd(nc, in_maps, core_ids=list(range(W)),
                                              trace=False)
    out = np.empty((N, C_OUT), np.float32)
    for c in range(W):
        out[c * NPC_R:(c + 1) * NPC_R] = res.results[c]["out_own"][:NPC_R, :C_OUT]
    out += np.outer(s1, b2)
    kernel._last = res
    return out


# revision 12
# speedup vs baseline: 1.5498x; 1.1686x over previous
"""LAHGCN hypergraph-conv kernel for 8 Trainium2 NeuronCores (bf16).

Math (per reference):
  smooth(x) = Dv^-1/2 H De^-1 H^T Dv^-1/2 x  (S),  branches k=0..3:
  hidden_k = relu(S(x_k W1_k + 1 b1_k));  out = concat(hidden) W2 + b2;  res = S out.

Key restructuring vs the padded-one-hot baseline:
  * W1 commutes with the edge aggregation: H^T(dv*(x_k W1_k)) = (H^T(dv*x_k)) W1_k,
    so we upload host-prescaled dv*x (bf16, replicated to every core) and gather
    x-rows directly -- no N-side x@W1 pass and no y AllGather.
    ef_k = de * (z_k W1_k + t b1_k) with z = H^T(dv x), t = H^T dv.
  * All four gather passes (edge-sorted x2, node-sorted x2) use
    dma_gather(prepare_only=True) + trigger_dma so the GpSimd engine only pays
    descriptor generation; transfers queue in the SWDGE ring and drain at SDMA
    rate, overlapped with the one-hot TensorE accumulation.
  * No dense H slabs: the second smooth (C_OUT padded 128) is pure gather too,
    reusing the same index/segment streams as the first smooth.
  * Degree scalings: dv folded into the uploaded x; de on the edge passes;
    dv^2 post-W2 (relu commutes with dv>=0); final dv on output;
    b2 via host-side rank-1 s1 = S@1 correction.
"""
import numpy as np
import ml_dtypes

BF16 = ml_dtypes.bfloat16
N, E, NNZ = 50000, 20000, 1600000
CONCAT, C_IN, C_HID = 4, 256, 256
C = CONCAT * C_HID            # 1024
C_OUT, C_OUT_P = 40, 128
W = 8
NPC_R, EPC_R = N // W, E // W           # 6250, 2500 real per core
NBLK, EBLK = 49, 20
NPC, EPC = NBLK * 128, EBLK * 128       # 6272, 2560 padded per core
NP_, EP_ = W * NPC, W * EPC             # 50176, 20480
NHALF = NP_ // 2                        # 25088 (int16 gather index split)
BATCH = 8                               # chunks per dma_gather (1024 idx max)
USE_PREP = False                        # prepare_only + trigger_dma pipelining
NQ = 4                                  # SWDGE queues used for gathers (ring
                                        # per queue -> overlapped transfers)


def _wrap_idx(idx):
    """[L] int -> [128, L/16] int16 wrapped layout, replicated across q7 cores."""
    L = len(idx)
    assert L % 16 == 0
    a = np.full((16, L // 16), 0, np.int16)
    a[np.arange(L) % 16, np.arange(L) // 16] = idx.astype(np.int16)
    return np.tile(a, (8, 1))


def _streams_var(rows, segpos, kbs):
    """Flat index stream + seg table with per-block chunk counts kbs."""
    total = sum(kbs)
    idx = np.zeros(total * 128, np.int64)
    seg = np.full((128, total), -1.0, np.float32)
    off = 0
    for r, p, kb in zip(rows, segpos, kbs):
        n = len(r)
        assert n <= kb * 128
        idx[off * 128:off * 128 + n] = r
        cols = off + np.arange(n) // 128
        seg[np.arange(n) % 128, cols] = p.astype(np.float32)
        off += kb
    return idx, seg.astype(BF16)


def _prep(node_idx, edge_idx, dv_is, de_inv, t_full):
    """Host-side prep: sorted gather streams + per-core scale tables."""
    nrow = (node_idx // NPC_R) * NPC + node_idx % NPC_R    # node -> padded row
    erow = (edge_idx // EPC_R) * EPC + edge_idx % EPC_R    # edge -> padded row
    p1 = np.argsort(edge_idx, kind="stable")
    e1, n1 = edge_idx[p1], nrow[p1]
    p2 = np.argsort(node_idx, kind="stable")
    n2, e2 = node_idx[p2], erow[p2]
    per = []
    for c in range(W):
        m1 = (e1 >= c * EPC_R) & (e1 < (c + 1) * EPC_R)
        el = e1[m1] - c * EPC_R
        nr = n1[m1]
        lo_rows, lo_pos, hi_rows, hi_pos = [], [], [], []
        for b in range(EBLK):
            mb = (el >= b * 128) & (el < (b + 1) * 128)
            rb, pb = nr[mb], el[mb] - b * 128
            lo = rb < NHALF
            lo_rows.append(rb[lo]); lo_pos.append(pb[lo])
            hi_rows.append(rb[~lo] - NHALF); hi_pos.append(pb[~lo])
        m2 = (n2 >= c * NPC_R) & (n2 < (c + 1) * NPC_R)
        nl = n2[m2] - c * NPC_R
        er = e2[m2]
        c_rows, c_pos = [], []
        for b in range(NBLK):
            mb = (nl >= b * 128) & (nl < (b + 1) * 128)
            c_rows.append(er[mb]); c_pos.append(nl[mb] - b * 128)
        per.append((lo_rows, lo_pos, hi_rows, hi_pos, c_rows, c_pos))
    kbA = [max(1, max((len(p[0][b]) + 127) // 128 for p in per)) for b in range(EBLK)]
    kbB = [max(1, max((len(p[2][b]) + 127) // 128 for p in per)) for b in range(EBLK)]
    kbC = [max(1, max((len(p[4][b]) + 127) // 128 for p in per)) for b in range(NBLK)]
    cores = []
    for c in range(W):
        lo_rows, lo_pos, hi_rows, hi_pos, c_rows, c_pos = per[c]
        iA, sA = _streams_var(lo_rows, lo_pos, kbA)
        iB, sB = _streams_var(hi_rows, hi_pos, kbB)
        iC, sC = _streams_var(c_rows, c_pos, kbC)
        dv = np.zeros(NPC, np.float32)
        dv[:NPC_R] = dv_is[c * NPC_R:(c + 1) * NPC_R]
        de = np.zeros(EPC, np.float32)
        de[:EPC_R] = de_inv[c * EPC_R:(c + 1) * EPC_R]
        t = np.zeros(EPC, np.float32)
        t[:EPC_R] = t_full[c * EPC_R:(c + 1) * EPC_R]
        cores.append(dict(
            idxA=_wrap_idx(iA), segA=sA, idxB=_wrap_idx(iB), segB=sB,
            idxC=_wrap_idx(iC), segC=sC,
            t_row=t.reshape(1, EPC).astype(BF16),
            dv_blk=dv.reshape(NBLK, 128).T.copy(),
            dvsq_blk=(dv * dv).reshape(NBLK, 128).T.copy(),
            de_blk=de.reshape(EBLK, 128).T.copy()))
    return cores, tuple(kbA), tuple(kbB), tuple(kbC)


def _build(kbA, kbB, kbC):
    import concourse.bass as bass
    import concourse.mybir as mybir
    from concourse import bacc, masks
    from concourse.tile import TileContext

    f32, bf16, i16 = mybir.dt.float32, mybir.dt.bfloat16, mybir.dt.int16
    sumA, sumB, sumC = sum(kbA), sum(kbB), sum(kbC)
    oA = np.concatenate([[0], np.cumsum(kbA)]).tolist()
    oB = np.concatenate([[0], np.cumsum(kbB)]).tolist()
    oC = np.concatenate([[0], np.cumsum(kbC)]).tolist()

    nc = bacc.Bacc("TRN2", num_devices=W, num_swdge_queues=NQ)
    T = lambda n, s, d=bf16: nc.dram_tensor(n, s, d, kind="ExternalInput")
    dvx = T("dvx", [NP_, C])                 # host-prescaled dv*x, branch-major
    W1 = T("W1", [CONCAT, C_IN, C_HID])
    b1c = T("b1c", [1, C])
    t_row_d = T("t_row", [1, EPC])
    W2p = T("W2p", [C, C_OUT_P])
    dv_blk = T("dv_blk", [128, NBLK], f32); dvsq_blk = T("dvsq_blk", [128, NBLK], f32)
    de_blk = T("de_blk", [128, EBLK], f32)
    idxA = T("idxA", [128, sumA * 8], i16); segA = T("segA", [128, sumA])
    idxB = T("idxB", [128, sumB * 8], i16); segB = T("segB", [128, sumB])
    idxC = T("idxC", [128, sumC * 8], i16); segC = T("segC", [128, sumC])
    iota_d = T("iota", [128, 128])
    out_own = nc.dram_tensor("out_own", [NPC, C_OUT_P], f32, kind="ExternalOutput")
    I = lambda n, s: nc.dram_tensor(n, s, bf16, kind="Internal")
    S = lambda n, s: nc.dram_tensor(n, s, bf16, kind="Internal", addr_space="Shared")
    ef_own, ef_full = I("ef_own", [EPC, C]), S("ef_full", [EP_, C])
    y2_own, y2_full = I("y2_own", [NPC, C_OUT_P]), S("y2_full", [NP_, C_OUT_P])
    ef2_own, ef2_full = I("ef2_own", [EPC, C_OUT_P]), S("ef2_full", [EP_, C_OUT_P])
    RG = [list(range(W))]
    AG = lambda i, o: nc.gpsimd.collective_compute(
        "AllGather", mybir.AluOpType.bypass, replica_groups=RG, ins=[i[:]], outs=[o[:]])

    with TileContext(nc) as tc:
        with tc.tile_pool(name="const", bufs=1) as cp:
            w1_sb = cp.tile([128, CONCAT * 2 * C_HID], bf16)     # f=(k*2+q) -> 256 cols
            for k in range(CONCAT):
                for q in range(2):
                    nc.sync.dma_start(
                        w1_sb[:, (k * 2 + q) * C_HID:(k * 2 + q + 1) * C_HID],
                        W1[k, q * 128:(q + 1) * 128, :])
            w2_sb = cp.tile([128, 8 * C_OUT_P], bf16)
            for f in range(8):
                nc.sync.dma_start(w2_sb[:, f * C_OUT_P:(f + 1) * C_OUT_P],
                                  W2p[f * 128:(f + 1) * 128, :])
            b1_sb = cp.tile([1, C], bf16); nc.sync.dma_start(b1_sb[:], b1c[:])
            t_sb = cp.tile([1, EPC], bf16); nc.sync.dma_start(t_sb[:], t_row_d[:])
            iota_sb = cp.tile([128, 128], bf16); nc.sync.dma_start(iota_sb[:], iota_d[:])
            ident = cp.tile([128, 128], bf16); masks.make_identity(nc, ident[:])
            dv_sb = cp.tile([128, NBLK], f32); nc.sync.dma_start(dv_sb[:], dv_blk[:])
            dvsq_sb = cp.tile([128, NBLK], f32); nc.sync.dma_start(dvsq_sb[:], dvsq_blk[:])
            de_sb = cp.tile([128, EBLK], f32); nc.sync.dma_start(de_sb[:], de_blk[:])
            iA = cp.tile([128, sumA * 8], i16); nc.sync.dma_start(iA[:], idxA[:])
            iB = cp.tile([128, sumB * 8], i16); nc.sync.dma_start(iB[:], idxB[:])
            iC = cp.tile([128, sumC * 8], i16); nc.scalar.dma_start(iC[:], idxC[:])
            sA = cp.tile([128, sumA], bf16); nc.scalar.dma_start(sA[:], segA[:])
            sB = cp.tile([128, sumB], bf16); nc.scalar.dma_start(sB[:], segB[:])
            sC = cp.tile([128, sumC], bf16); nc.scalar.dma_start(sC[:], segC[:])

            mm = lambda *a, **kw: nc.tensor.matmul(*a, skip_group_check=True, **kw)
            qsems = [nc.alloc_semaphore(f"gq{q}") for q in range(NQ)] if USE_PREP else None
            qctr = [0]
            qthr = [0] * NQ
            if USE_PREP:
                for q in range(NQ):
                    nc.gpsimd.sem_clear(qsems[q])

            def seg_pass(kb, off, idx_sb, seg_sb, src_ap, elem, pool, ps,
                         start_stream, stop_stream, tag):
                """Gather + one-hot-matmul accumulation for one block's stream."""
                for s in range(0, kb, BATCH):
                    nch = min(BATCH, kb - s)
                    k0 = off + s
                    g = pool.tile([128, BATCH, elem], bf16, tag=tag + "g")
                    gate = None
                    if USE_PREP:
                        q = qctr[0] % NQ
                        qctr[0] += 1
                        nc.gpsimd.dma_gather(
                            out_ap=g[:, :nch, :], in_ap=src_ap,
                            idxs_ap=idx_sb[:, k0 * 8:(k0 + nch) * 8],
                            num_idxs=nch * 128, num_idxs_reg=nch * 128,
                            elem_size=elem, prepare_only=True, sem=qsems[q],
                            queue_num=q)
                        nc.gpsimd.trigger_dma(count=None, queue_num=q)
                        qthr[q] += 16
                        gate = (qsems[q], qthr[q])
                    else:
                        q = qctr[0] % NQ
                        qctr[0] += 1
                        nc.gpsimd.dma_gather(
                            out_ap=g[:, :nch, :], in_ap=src_ap,
                            idxs_ap=idx_sb[:, k0 * 8:(k0 + nch) * 8],
                            num_idxs=nch * 128, num_idxs_reg=nch * 128,
                            elem_size=elem, queue_num=q)
                    oh = pool.tile([128, BATCH, 128], bf16, tag=tag + "o")
                    nc.vector.tensor_tensor(
                        out=oh[:, :nch, :],
                        in0=iota_sb[:, None, :].broadcast_to([128, nch, 128]),
                        in1=seg_sb[:, k0:k0 + nch, None].broadcast_to([128, nch, 128]),
                        op=mybir.AluOpType.is_equal)
                    if gate is not None:
                        nc.tensor.wait_ge(gate[0], gate[1])
                    for j in range(nch):
                        first = start_stream and (s == 0 and j == 0)
                        last = stop_stream and (s + j == kb - 1)
                        for h in range((elem + 511) // 512):
                            w_ = min(512, elem - h * 512)
                            mm(ps[:, h * 512:h * 512 + w_],
                               lhsT=oh[:, j, :], rhs=g[:, j, h * 512:h * 512 + w_],
                               start=first, stop=last)

            # ---- phase B': z = H^T(dv x); ef = de * (z_k W1_k + t b1_k) ----
            with tc.tile_pool(name="pb", bufs=3) as pb, \
                 tc.tile_pool(name="pbz", bufs=2, space="PSUM") as pbz, \
                 tc.tile_pool(name="pbt", bufs=1, space="PSUM") as pbt, \
                 tc.tile_pool(name="pbe", bufs=1, space="PSUM") as pbe:
                for b in range(EBLK):
                    pz = pbz.tile([128, C], f32, tag="pz")
                    seg_pass(kbA[b], oA[b], iA, sA, dvx[0:NHALF, :], C,
                             pb, pz, True, False, "A")
                    seg_pass(kbB[b], oB[b], iB, sB, dvx[NHALF:NP_, :], C,
                             pb, pz, False, True, "B")
                    z_sb = pb.tile([128, C], bf16, tag="zsb")
                    nc.vector.tensor_copy(z_sb[:], pz[:])
                    pt = pbt.tile([128, C], bf16, tag="pt")
                    for f in range(8):
                        nc.tensor.transpose(pt[:, f * 128:(f + 1) * 128],
                                            z_sb[:, f * 128:(f + 1) * 128], ident[:])
                    zt_sb = pb.tile([128, C], bf16, tag="ztsb")
                    nc.vector.tensor_copy(zt_sb[:], pt[:])
                    pef = pbe.tile([128, C], f32, tag="pef")
                    tb = t_sb[:, b * 128:(b + 1) * 128]
                    mm(pef[:, :512], lhsT=tb, rhs=b1_sb[:, :512], start=True, stop=False)
                    mm(pef[:, 512:], lhsT=tb, rhs=b1_sb[:, 512:], start=True, stop=False)
                    for k in range(CONCAT):
                        for qh in range(2):
                            f = 2 * k + qh
                            mm(pef[:, k * C_HID:(k + 1) * C_HID],
                               lhsT=zt_sb[:, f * 128:(f + 1) * 128],
                               rhs=w1_sb[:, f * C_HID:(f + 1) * C_HID],
                               start=False, stop=(qh == 1))
                    ef_sb = pb.tile([128, C], bf16, tag="efsb")
                    nc.vector.tensor_tensor(
                        out=ef_sb[:], in0=pef[:],
                        in1=de_sb[:, b:b + 1].broadcast_to([128, C]),
                        op=mybir.AluOpType.mult)
                    nc.sync.dma_start(ef_own[b * 128:(b + 1) * 128, :], ef_sb[:])
            AG(ef_own, ef_full)

            # ---- phase C: u = relu(H ef); y2 = dv^2 * (u @ W2) ----
            with tc.tile_pool(name="pc", bufs=3) as pc, \
                 tc.tile_pool(name="pcz", bufs=2, space="PSUM") as pcz, \
                 tc.tile_pool(name="pct", bufs=1, space="PSUM") as pct, \
                 tc.tile_pool(name="pcy", bufs=2, space="PSUM") as pcy:
                for b in range(NBLK):
                    pu = pcz.tile([128, C], f32, tag="pu")
                    seg_pass(kbC[b], oC[b], iC, sC, ef_full[:], C, pc, pu,
                             True, True, "C")
                    u_sb = pc.tile([128, C], bf16, tag="usb")
                    nc.scalar.activation(out=u_sb[:], in_=pu[:],
                                         func=mybir.ActivationFunctionType.Relu)
                    pt = pct.tile([128, C], bf16, tag="ptc")
                    for f in range(8):
                        nc.tensor.transpose(pt[:, f * 128:(f + 1) * 128],
                                            u_sb[:, f * 128:(f + 1) * 128], ident[:])
                    ut_sb = pc.tile([128, C], bf16, tag="utsb")
                    nc.vector.tensor_copy(ut_sb[:], pt[:])
                    py2 = pcy.tile([128, C_OUT_P], f32, tag="py2")
                    for f in range(8):
                        mm(py2[:], lhsT=ut_sb[:, f * 128:(f + 1) * 128],
                           rhs=w2_sb[:, f * C_OUT_P:(f + 1) * C_OUT_P],
                           start=(f == 0), stop=(f == 7))
                    y2_sb = pc.tile([128, C_OUT_P], bf16, tag="y2sb")
                    nc.vector.tensor_tensor(
                        out=y2_sb[:], in0=py2[:],
                        in1=dvsq_sb[:, b:b + 1].broadcast_to([128, C_OUT_P]),
                        op=mybir.AluOpType.mult)
                    nc.sync.dma_start(y2_own[b * 128:(b + 1) * 128, :], y2_sb[:])
            AG(y2_own, y2_full)

            # ---- phase D: ef2 = de * (H^T y2), same streams as B' ----
            with tc.tile_pool(name="pd", bufs=6) as pd, \
                 tc.tile_pool(name="pdp", bufs=4, space="PSUM") as pdp:
                for b in range(EBLK):
                    ps2 = pdp.tile([128, C_OUT_P], f32, tag="ps2")
                    seg_pass(kbA[b], oA[b], iA, sA, y2_full[0:NHALF, :],
                             C_OUT_P, pd, ps2, True, False, "D")
                    seg_pass(kbB[b], oB[b], iB, sB, y2_full[NHALF:NP_, :],
                             C_OUT_P, pd, ps2, False, True, "E")
                    e2_sb = pd.tile([128, C_OUT_P], bf16, tag="e2sb")
                    nc.vector.tensor_tensor(
                        out=e2_sb[:], in0=ps2[:],
                        in1=de_sb[:, b:b + 1].broadcast_to([128, C_OUT_P]),
                        op=mybir.AluOpType.mult)
                    nc.sync.dma_start(ef2_own[b * 128:(b + 1) * 128, :], e2_sb[:])
            AG(ef2_own, ef2_full)

            # ---- phase E: res = dv * (H ef2), same stream as C ----
            with tc.tile_pool(name="pe", bufs=6) as pe_, \
                 tc.tile_pool(name="pep", bufs=4, space="PSUM") as pep:
                for b in range(NBLK):
                    po = pep.tile([128, C_OUT_P], f32, tag="po")
                    seg_pass(kbC[b], oC[b], iC, sC, ef2_full[:], C_OUT_P,
                             pe_, po, True, True, "F")
                    o_sb = pe_.tile([128, C_OUT_P], f32, tag="osb")
                    nc.vector.tensor_tensor(
                        out=o_sb[:], in0=po[:],
                        in1=dv_sb[:, b:b + 1].broadcast_to([128, C_OUT_P]),
                        op=mybir.AluOpType.mult)
                    nc.sync.dma_start(out_own[b * 128:(b + 1) * 128, :], o_sb[:])
    nc.finalize()
    return nc


_CACHE = {}


def kernel(x_list, W1, b1, W2, b2, node_idx, edge_idx, n_edges, _trace=False,
           _tmpdir=None):
    from concourse import bass_utils
    x_list = np.asarray(x_list, np.float32); W1 = np.asarray(W1, np.float32)
    b1 = np.asarray(b1, np.float32); W2 = np.asarray(W2, np.float32)
    b2 = np.asarray(b2, np.float32)
    node_idx = np.asarray(node_idx, np.int32); edge_idx = np.asarray(edge_idx, np.int32)

    dv = np.bincount(node_idx, minlength=N).astype(np.float32)
    de = np.bincount(edge_idx, minlength=E).astype(np.float32)
    dv_is = np.where(dv > 0, 1.0 / np.sqrt(np.maximum(dv, 1.0)), 0.0).astype(np.float32)
    de_inv = np.where(de > 0, 1.0 / np.maximum(de, 1.0), 0.0).astype(np.float32)
    # t = H^T dv (edge sums of dv) for the b1 rank-1 term
    t_full = np.bincount(edge_idx, weights=dv_is[node_idx], minlength=E).astype(np.float32)
    # s1 = S @ 1 for the host-side b2 rank-1 term
    ef_t = t_full * de_inv
    s1 = dv_is * np.bincount(node_idx, weights=ef_t[edge_idx], minlength=N)

    cores, kbA, kbB, kbC = _prep(node_idx, edge_idx, dv_is, de_inv, t_full)
    key = (kbA, kbB, kbC)
    if key not in _CACHE:
        _CACHE[key] = _build(kbA, kbB, kbC)
    nc = _CACHE[key]

    W2p = np.zeros((C, C_OUT_P), np.float32)
    W2p[:, :C_OUT] = W2
    iota_np = np.tile(np.arange(128, dtype=np.float32), (128, 1))
    # dvx[padded row, (k,cin)] = dv[n] * x[k, n, cin], branch-major columns
    dvx_real = (x_list.transpose(1, 0, 2).reshape(N, C)
                * dv_is[:, None]).astype(BF16)
    dvx_p = np.zeros((NP_, C), BF16)
    for c in range(W):
        dvx_p[c * NPC:c * NPC + NPC_R] = dvx_real[c * NPC_R:(c + 1) * NPC_R]
    in_maps = []
    for c in range(W):
        m = dict(dvx=dvx_p, W1=W1.astype(BF16),
                 b1c=b1.reshape(1, C).astype(BF16), W2p=W2p.astype(BF16),
                 iota=iota_np.astype(BF16), **cores[c])
        in_maps.append(m)
    try:
        res = bass_utils.run_bass_kernel_spmd(nc, in_maps, core_ids=list(range(W)),
                                              trace=_trace, tmpdir=_tmpdir)
    except ModuleNotFoundError:
        res = bass_utils.run_bass_kernel_spmd(nc, in_maps, core_ids=list(range(W)),
                                              trace=False)
    out = np.empty((N, C_OUT), np.float32)
    for c in range(W):
        out[c * NPC_R:(c + 1) * NPC_R] = res.results[c]["out_own"][:NPC_R, :C_OUT]
    out += np.outer(s1, b2)
    kernel._last = res
    return out


# revision 16
# speedup vs baseline: 1.6484x; 1.0636x over previous
"""LAHGCN hypergraph-conv kernel for 8 Trainium2 NeuronCores (bf16).

Math (per reference):
  smooth(x) = Dv^-1/2 H De^-1 H^T Dv^-1/2 x  (S),  branches k=0..3:
  hidden_k = relu(S(x_k W1_k + 1 b1_k));  out = concat(hidden) W2 + b2;  res = S out.

Key restructuring vs the padded-one-hot baseline:
  * W1 commutes with the edge aggregation: H^T(dv*(x_k W1_k)) = (H^T(dv*x_k)) W1_k,
    so we upload host-prescaled dv*x (bf16, replicated to every core) and gather
    x-rows directly -- no N-side x@W1 pass and no y AllGather.
    ef_k = de * (z_k W1_k + t b1_k) with z = H^T(dv x), t = H^T dv.
  * All four gather passes (edge-sorted x2, node-sorted x2) use
    dma_gather(prepare_only=True) + trigger_dma so the GpSimd engine only pays
    descriptor generation; transfers queue in the SWDGE ring and drain at SDMA
    rate, overlapped with the one-hot TensorE accumulation.
  * No dense H slabs: the second smooth (C_OUT padded 128) is pure gather too,
    reusing the same index/segment streams as the first smooth.
  * Degree scalings: dv folded into the uploaded x; de on the edge passes;
    dv^2 post-W2 (relu commutes with dv>=0); final dv on output;
    b2 via host-side rank-1 s1 = S@1 correction.
"""
import numpy as np
import ml_dtypes

BF16 = ml_dtypes.bfloat16
N, E, NNZ = 50000, 20000, 1600000
CONCAT, C_IN, C_HID = 4, 256, 256
C = CONCAT * C_HID            # 1024
C_OUT, C_OUT_P = 40, 128
W = 8
NPC_R, EPC_R = N // W, E // W           # 6250, 2500 real per core
NBLK, EBLK = 49, 20
NPC, EPC = NBLK * 128, EBLK * 128       # 6272, 2560 padded per core
NP_, EP_ = W * NPC, W * EPC             # 50176, 20480
NHALF = NP_ // 2                        # 25088 (int16 gather index split)
BATCH = 8                               # chunks per dma_gather (1024 idx max)
USE_PREP = False                        # prepare_only + trigger_dma pipelining
NQ = 4                                  # SWDGE queues used for gathers (ring
                                        # per queue -> overlapped transfers)


def _wrap_idx(idx):
    """[L] int -> [128, L/16] int16 wrapped layout, replicated across q7 cores."""
    L = len(idx)
    assert L % 16 == 0
    a = np.full((16, L // 16), 0, np.int16)
    a[np.arange(L) % 16, np.arange(L) // 16] = idx.astype(np.int16)
    return np.tile(a, (8, 1))


def _streams_var(rows, segpos, kbs):
    """Flat index stream + seg table with per-block chunk counts kbs."""
    total = sum(kbs)
    idx = np.zeros(total * 128, np.int64)
    seg = np.full((128, total), -1.0, np.float32)
    off = 0
    for r, p, kb in zip(rows, segpos, kbs):
        n = len(r)
        assert n <= kb * 128
        idx[off * 128:off * 128 + n] = r
        cols = off + np.arange(n) // 128
        seg[np.arange(n) % 128, cols] = p.astype(np.float32)
        off += kb
    return idx, seg.astype(BF16)


def _prep(node_idx, edge_idx, dv_is, de_inv, t_full):
    """Host-side prep: sorted gather streams + per-core scale tables."""
    nrow = (node_idx // NPC_R) * NPC + node_idx % NPC_R    # node -> padded row
    erow = (edge_idx // EPC_R) * EPC + edge_idx % EPC_R    # edge -> padded row
    p1 = np.argsort(edge_idx, kind="stable")
    e1, n1 = edge_idx[p1], nrow[p1]
    p2 = np.argsort(node_idx, kind="stable")
    n2, e2 = node_idx[p2], erow[p2]
    per = []
    for c in range(W):
        m1 = (e1 >= c * EPC_R) & (e1 < (c + 1) * EPC_R)
        el = e1[m1] - c * EPC_R
        nr = n1[m1]
        lo_rows, lo_pos, hi_rows, hi_pos = [], [], [], []
        for b in range(EBLK):
            mb = (el >= b * 128) & (el < (b + 1) * 128)
            rb, pb = nr[mb], el[mb] - b * 128
            lo = rb < NHALF
            lo_rows.append(rb[lo]); lo_pos.append(pb[lo])
            hi_rows.append(rb[~lo] - NHALF); hi_pos.append(pb[~lo])
        m2 = (n2 >= c * NPC_R) & (n2 < (c + 1) * NPC_R)
        nl = n2[m2] - c * NPC_R
        er = e2[m2]
        c_rows, c_pos = [], []
        for b in range(NBLK):
            mb = (nl >= b * 128) & (nl < (b + 1) * 128)
            c_rows.append(er[mb]); c_pos.append(nl[mb] - b * 128)
        per.append((lo_rows, lo_pos, hi_rows, hi_pos, c_rows, c_pos))
    kbA = [max(1, max((len(p[0][b]) + 127) // 128 for p in per)) for b in range(EBLK)]
    kbB = [max(1, max((len(p[2][b]) + 127) // 128 for p in per)) for b in range(EBLK)]
    kbC = [max(1, max((len(p[4][b]) + 127) // 128 for p in per)) for b in range(NBLK)]
    cores = []
    for c in range(W):
        lo_rows, lo_pos, hi_rows, hi_pos, c_rows, c_pos = per[c]
        iA, sA = _streams_var(lo_rows, lo_pos, kbA)
        iB, sB = _streams_var(hi_rows, hi_pos, kbB)
        iC, sC = _streams_var(c_rows, c_pos, kbC)
        dv = np.zeros(NPC, np.float32)
        dv[:NPC_R] = dv_is[c * NPC_R:(c + 1) * NPC_R]
        de = np.zeros(EPC, np.float32)
        de[:EPC_R] = de_inv[c * EPC_R:(c + 1) * EPC_R]
        t = np.zeros(EPC, np.float32)
        t[:EPC_R] = t_full[c * EPC_R:(c + 1) * EPC_R]
        cores.append(dict(
            idxA=_wrap_idx(iA), segA=sA, idxB=_wrap_idx(iB), segB=sB,
            idxC=_wrap_idx(iC), segC=sC,
            t_row=t.reshape(1, EPC).astype(BF16),
            dv_blk=dv.reshape(NBLK, 128).T.copy(),
            dvsq_blk=(dv * dv).reshape(NBLK, 128).T.copy(),
            de_blk=de.reshape(EBLK, 128).T.copy()))
    return cores, tuple(kbA), tuple(kbB), tuple(kbC)


def _build(kbA, kbB, kbC):
    import concourse.bass as bass
    import concourse.mybir as mybir
    from concourse import bacc, masks
    from concourse.tile import TileContext

    f32, bf16, i16 = mybir.dt.float32, mybir.dt.bfloat16, mybir.dt.int16
    sumA, sumB, sumC = sum(kbA), sum(kbB), sum(kbC)
    oA = np.concatenate([[0], np.cumsum(kbA)]).tolist()
    oB = np.concatenate([[0], np.cumsum(kbB)]).tolist()
    oC = np.concatenate([[0], np.cumsum(kbC)]).tolist()

    nc = bacc.Bacc("TRN2", num_devices=W, num_swdge_queues=NQ)
    T = lambda n, s, d=bf16: nc.dram_tensor(n, s, d, kind="ExternalInput")
    dvx = T("dvx", [NP_, C])                 # host-prescaled dv*x, branch-major
    W1 = T("W1", [CONCAT, C_IN, C_HID])
    b1c = T("b1c", [1, C])
    t_row_d = T("t_row", [1, EPC])
    W2p = T("W2p", [C, C_OUT_P])
    dv_blk = T("dv_blk", [128, NBLK], f32); dvsq_blk = T("dvsq_blk", [128, NBLK], f32)
    de_blk = T("de_blk", [128, EBLK], f32)
    idxA = T("idxA", [128, sumA * 8], i16); segA = T("segA", [128, sumA])
    idxB = T("idxB", [128, sumB * 8], i16); segB = T("segB", [128, sumB])
    idxC = T("idxC", [128, sumC * 8], i16); segC = T("segC", [128, sumC])
    iota_d = T("iota", [128, 128])
    out_own = nc.dram_tensor("out_own", [NPC, C_OUT_P], f32, kind="ExternalOutput")
    I = lambda n, s: nc.dram_tensor(n, s, bf16, kind="Internal")
    S = lambda n, s: nc.dram_tensor(n, s, bf16, kind="Internal", addr_space="Shared")
    ef_own, ef_full = I("ef_own", [EPC, C]), S("ef_full", [EP_, C])
    y2_own, y2_full = I("y2_own", [NPC, C_OUT_P]), S("y2_full", [NP_, C_OUT_P])
    ef2_own, ef2_full = I("ef2_own", [EPC, C_OUT_P]), S("ef2_full", [EP_, C_OUT_P])
    RG = [list(range(W))]
    AG = lambda i, o: nc.gpsimd.collective_compute(
        "AllGather", mybir.AluOpType.bypass, replica_groups=RG, ins=[i[:]], outs=[o[:]])

    with TileContext(nc) as tc:
        with tc.tile_pool(name="const", bufs=1) as cp:
            w1_sb = cp.tile([128, CONCAT * 2 * C_HID], bf16)     # f=(k*2+q) -> 256 cols
            for k in range(CONCAT):
                for q in range(2):
                    nc.sync.dma_start(
                        w1_sb[:, (k * 2 + q) * C_HID:(k * 2 + q + 1) * C_HID],
                        W1[k, q * 128:(q + 1) * 128, :])
            w2_sb = cp.tile([128, 8 * C_OUT_P], bf16)
            for f in range(8):
                nc.sync.dma_start(w2_sb[:, f * C_OUT_P:(f + 1) * C_OUT_P],
                                  W2p[f * 128:(f + 1) * 128, :])
            b1_sb = cp.tile([1, C], bf16); nc.sync.dma_start(b1_sb[:], b1c[:])
            t_sb = cp.tile([1, EPC], bf16); nc.sync.dma_start(t_sb[:], t_row_d[:])
            iota_sb = cp.tile([128, 128], bf16); nc.sync.dma_start(iota_sb[:], iota_d[:])
            ident = cp.tile([128, 128], bf16); masks.make_identity(nc, ident[:])
            dv_sb = cp.tile([128, NBLK], f32); nc.sync.dma_start(dv_sb[:], dv_blk[:])
            dvsq_sb = cp.tile([128, NBLK], f32); nc.sync.dma_start(dvsq_sb[:], dvsq_blk[:])
            de_sb = cp.tile([128, EBLK], f32); nc.sync.dma_start(de_sb[:], de_blk[:])
            iA = cp.tile([128, sumA * 8], i16); nc.sync.dma_start(iA[:], idxA[:])
            iB = cp.tile([128, sumB * 8], i16); nc.sync.dma_start(iB[:], idxB[:])
            iC = cp.tile([128, sumC * 8], i16); nc.scalar.dma_start(iC[:], idxC[:])
            sA = cp.tile([128, sumA], bf16); nc.scalar.dma_start(sA[:], segA[:])
            sB = cp.tile([128, sumB], bf16); nc.scalar.dma_start(sB[:], segB[:])
            sC = cp.tile([128, sumC], bf16); nc.scalar.dma_start(sC[:], segC[:])

            mm = lambda *a, **kw: nc.tensor.matmul(*a, skip_group_check=True, **kw)
            qsems = [nc.alloc_semaphore(f"gq{q}") for q in range(NQ)] if USE_PREP else None
            qctr = [0]
            qthr = [0] * NQ
            if USE_PREP:
                for q in range(NQ):
                    nc.gpsimd.sem_clear(qsems[q])

            def seg_pass(kb, off, idx_sb, seg_sb, src_ap, elem, pool, ps,
                         start_stream, stop_stream, tag):
                """Gather + one-hot-matmul accumulation for one block's stream."""
                for s in range(0, kb, BATCH):
                    nch = min(BATCH, kb - s)
                    k0 = off + s
                    g = pool.tile([128, BATCH, elem], bf16, tag=tag + "g")
                    gate = None
                    if USE_PREP:
                        q = qctr[0] % NQ
                        qctr[0] += 1
                        nc.gpsimd.dma_gather(
                            out_ap=g[:, :nch, :], in_ap=src_ap,
                            idxs_ap=idx_sb[:, k0 * 8:(k0 + nch) * 8],
                            num_idxs=nch * 128, num_idxs_reg=nch * 128,
                            elem_size=elem, prepare_only=True, sem=qsems[q],
                            queue_num=q)
                        nc.gpsimd.trigger_dma(count=None, queue_num=q)
                        qthr[q] += 16
                        gate = (qsems[q], qthr[q])
                    else:
                        q = qctr[0] % NQ
                        qctr[0] += 1
                        nc.gpsimd.dma_gather(
                            out_ap=g[:, :nch, :], in_ap=src_ap,
                            idxs_ap=idx_sb[:, k0 * 8:(k0 + nch) * 8],
                            num_idxs=nch * 128, num_idxs_reg=nch * 128,
                            elem_size=elem, queue_num=q)
                    oh = pool.tile([128, BATCH, 128], bf16, tag=tag + "o")
                    nc.vector.tensor_tensor(
                        out=oh[:, :nch, :],
                        in0=iota_sb[:, None, :].broadcast_to([128, nch, 128]),
                        in1=seg_sb[:, k0:k0 + nch, None].broadcast_to([128, nch, 128]),
                        op=mybir.AluOpType.is_equal)
                    if gate is not None:
                        nc.tensor.wait_ge(gate[0], gate[1])
                    for j in range(nch):
                        first = start_stream and (s == 0 and j == 0)
                        last = stop_stream and (s + j == kb - 1)
                        for h in range((elem + 511) // 512):
                            w_ = min(512, elem - h * 512)
                            mm(ps[:, h * 512:h * 512 + w_],
                               lhsT=oh[:, j, :], rhs=g[:, j, h * 512:h * 512 + w_],
                               start=first, stop=last)

            # ---- phase B': z = H^T(dv x); ef = de * (z_k W1_k + t b1_k) ----
            with tc.tile_pool(name="pb", bufs=4) as pb, \
                 tc.tile_pool(name="pbz", bufs=2, space="PSUM") as pbz, \
                 tc.tile_pool(name="pbt", bufs=1, space="PSUM") as pbt, \
                 tc.tile_pool(name="pbe", bufs=1, space="PSUM") as pbe:
                for b in range(EBLK):
                    pz = pbz.tile([128, C], f32, tag="pz")
                    seg_pass(kbA[b], oA[b], iA, sA, dvx[0:NHALF, :], C,
                             pb, pz, True, False, "A")
                    seg_pass(kbB[b], oB[b], iB, sB, dvx[NHALF:NP_, :], C,
                             pb, pz, False, True, "A")
                    z_sb = pb.tile([128, C], bf16, tag="zsb")
                    nc.vector.tensor_copy(z_sb[:], pz[:])
                    pt = pbt.tile([128, C], bf16, tag="pt")
                    for f in range(8):
                        nc.tensor.transpose(pt[:, f * 128:(f + 1) * 128],
                                            z_sb[:, f * 128:(f + 1) * 128], ident[:])
                    zt_sb = pb.tile([128, C], bf16, tag="ztsb")
                    nc.vector.tensor_copy(zt_sb[:], pt[:])
                    pef = pbe.tile([128, C], f32, tag="pef")
                    tb = t_sb[:, b * 128:(b + 1) * 128]
                    mm(pef[:, :512], lhsT=tb, rhs=b1_sb[:, :512], start=True, stop=False)
                    mm(pef[:, 512:], lhsT=tb, rhs=b1_sb[:, 512:], start=True, stop=False)
                    for k in range(CONCAT):
                        for qh in range(2):
                            f = 2 * k + qh
                            mm(pef[:, k * C_HID:(k + 1) * C_HID],
                               lhsT=zt_sb[:, f * 128:(f + 1) * 128],
                               rhs=w1_sb[:, f * C_HID:(f + 1) * C_HID],
                               start=False, stop=(qh == 1))
                    ef_sb = pb.tile([128, C], bf16, tag="efsb")
                    nc.vector.tensor_tensor(
                        out=ef_sb[:], in0=pef[:],
                        in1=de_sb[:, b:b + 1].broadcast_to([128, C]),
                        op=mybir.AluOpType.mult)
                    nc.sync.dma_start(ef_own[b * 128:(b + 1) * 128, :], ef_sb[:])
            AG(ef_own, ef_full)

            # ---- phase C: u = relu(H ef); y2 = dv^2 * (u @ W2) ----
            with tc.tile_pool(name="pc", bufs=4) as pc, \
                 tc.tile_pool(name="pcz", bufs=3, space="PSUM") as pcz, \
                 tc.tile_pool(name="pct", bufs=1, space="PSUM") as pct, \
                 tc.tile_pool(name="pcy", bufs=1, space="PSUM") as pcy:
                for b in range(NBLK):
                    pu = pcz.tile([128, C], f32, tag="pu")
                    seg_pass(kbC[b], oC[b], iC, sC, ef_full[:], C, pc, pu,
                             True, True, "C")
                    u_sb = pc.tile([128, C], bf16, tag="usb")
                    nc.scalar.activation(out=u_sb[:], in_=pu[:],
                                         func=mybir.ActivationFunctionType.Relu)
                    pt = pct.tile([128, C], bf16, tag="ptc")
                    for f in range(8):
                        nc.tensor.transpose(pt[:, f * 128:(f + 1) * 128],
                                            u_sb[:, f * 128:(f + 1) * 128], ident[:])
                    ut_sb = pc.tile([128, C], bf16, tag="utsb")
                    nc.vector.tensor_copy(ut_sb[:], pt[:])
                    py2 = pcy.tile([128, C_OUT_P], f32, tag="py2")
                    for f in range(8):
                        mm(py2[:], lhsT=ut_sb[:, f * 128:(f + 1) * 128],
                           rhs=w2_sb[:, f * C_OUT_P:(f + 1) * C_OUT_P],
                           start=(f == 0), stop=(f == 7))
                    y2_sb = pc.tile([128, C_OUT_P], bf16, tag="y2sb")
                    nc.vector.tensor_tensor(
                        out=y2_sb[:], in0=py2[:],
                        in1=dvsq_sb[:, b:b + 1].broadcast_to([128, C_OUT_P]),
                        op=mybir.AluOpType.mult)
                    nc.sync.dma_start(y2_own[b * 128:(b + 1) * 128, :], y2_sb[:])
            AG(y2_own, y2_full)

            # ---- phase D: ef2 = de * (H^T y2), same streams as B' ----
            with tc.tile_pool(name="pd", bufs=6) as pd, \
                 tc.tile_pool(name="pdp", bufs=4, space="PSUM") as pdp:
                for b in range(EBLK):
                    ps2 = pdp.tile([128, C_OUT_P], f32, tag="ps2")
                    seg_pass(kbA[b], oA[b], iA, sA, y2_full[0:NHALF, :],
                             C_OUT_P, pd, ps2, True, False, "D")
                    seg_pass(kbB[b], oB[b], iB, sB, y2_full[NHALF:NP_, :],
                             C_OUT_P, pd, ps2, False, True, "D")
                    e2_sb = pd.tile([128, C_OUT_P], bf16, tag="e2sb")
                    nc.vector.tensor_tensor(
                        out=e2_sb[:], in0=ps2[:],
                        in1=de_sb[:, b:b + 1].broadcast_to([128, C_OUT_P]),
                        op=mybir.AluOpType.mult)
                    nc.sync.dma_start(ef2_own[b * 128:(b + 1) * 128, :], e2_sb[:])
            AG(ef2_own, ef2_full)

            # ---- phase E: res = dv * (H ef2), same stream as C ----
            with tc.tile_pool(name="pe", bufs=6) as pe_, \
                 tc.tile_pool(name="pep", bufs=4, space="PSUM") as pep:
                for b in range(NBLK):
                    po = pep.tile([128, C_OUT_P], f32, tag="po")
                    seg_pass(kbC[b], oC[b], iC, sC, ef2_full[:], C_OUT_P,
                             pe_, po, True, True, "F")
                    o_sb = pe_.tile([128, C_OUT_P], f32, tag="osb")
                    nc.vector.tensor_tensor(
                        out=o_sb[:], in0=po[:],
                        in1=dv_sb[:, b:b + 1].broadcast_to([128, C_OUT_P]),
                        op=mybir.AluOpType.mult)
                    nc.sync.dma_start(out_own[b * 128:(b + 1) * 128, :], o_sb[:])
    nc.finalize()
    return nc


_CACHE = {}


def kernel(x_list, W1, b1, W2, b2, node_idx, edge_idx, n_edges, _trace=False,
           _tmpdir=None):
    from concourse import bass_utils
    x_list = np.asarray(x_list, np.float32); W1 = np.asarray(W1, np.float32)
    b1 = np.asarray(b1, np.float32); W2 = np.asarray(W2, np.float32)
    b2 = np.asarray(b2, np.float32)
    node_idx = np.asarray(node_idx, np.int32); edge_idx = np.asarray(edge_idx, np.int32)

    dv = np.bincount(node_idx, minlength=N).astype(np.float32)
    de = np.bincount(edge_idx, minlength=E).astype(np.float32)
    dv_is = np.where(dv > 0, 1.0 / np.sqrt(np.maximum(dv, 1.0)), 0.0).astype(np.float32)
    de_inv = np.where(de > 0, 1.0 / np.maximum(de, 1.0), 0.0).astype(np.float32)
    # t = H^T dv (edge sums of dv) for the b1 rank-1 term
    t_full = np.bincount(edge_idx, weights=dv_is[node_idx], minlength=E).astype(np.float32)
    # s1 = S @ 1 for the host-side b2 rank-1 term
    ef_t = t_full * de_inv
    s1 = dv_is * np.bincount(node_idx, weights=ef_t[edge_idx], minlength=N)

    cores, kbA, kbB, kbC = _prep(node_idx, edge_idx, dv_is, de_inv, t_full)
    key = (kbA, kbB, kbC)
    if key not in _CACHE:
        _CACHE[key] = _build(kbA, kbB, kbC)
    nc = _CACHE[key]

    W2p = np.zeros((C, C_OUT_P), np.float32)
    W2p[:, :C_OUT] = W2
    iota_np = np.tile(np.arange(128, dtype=np.float32), (128, 1))
    # dvx[padded row, (k,cin)] = dv[n] * x[k, n, cin], branch-major columns
    dvx_real = (x_list.transpose(1, 0, 2).reshape(N, C)
                * dv_is[:, None]).astype(BF16)
    dvx_p = np.zeros((NP_, C), BF16)
    for c in range(W):
        dvx_p[c * NPC:c * NPC + NPC_R] = dvx_real[c * NPC_R:(c + 1) * NPC_R]
    in_maps = []
    for c in range(W):
        m = dict(dvx=dvx_p, W1=W1.astype(BF16),
                 b1c=b1.reshape(1, C).astype(BF16), W2p=W2p.astype(BF16),
                 iota=iota_np.astype(BF16), **cores[c])
        in_maps.append(m)
    try:
        res = bass_utils.run_bass_kernel_spmd(nc, in_maps, core_ids=list(range(W)),
                                              trace=_trace, tmpdir=_tmpdir)
    except ModuleNotFoundError:
        res = bass_utils.run_bass_kernel_spmd(nc, in_maps, core_ids=list(range(W)),
                                              trace=False)
    out = np.empty((N, C_OUT), np.float32)
    for c in range(W):
        out[c * NPC_R:(c + 1) * NPC_R] = res.results[c]["out_own"][:NPC_R, :C_OUT]
    out += np.outer(s1, b2)
    kernel._last = res
    return out


# revision 21
# speedup vs baseline: 1.7640x; 1.0701x over previous
"""LAHGCN hypergraph-conv kernel for 8 Trainium2 NeuronCores (bf16).

Math (per reference):
  smooth(x) = Dv^-1/2 H De^-1 H^T Dv^-1/2 x  (S),  branches k=0..3:
  hidden_k = relu(S(x_k W1_k + 1 b1_k));  out = concat(hidden) W2 + b2;  res = S out.

Key restructuring vs the padded-one-hot baseline:
  * W1 commutes with the edge aggregation: H^T(dv*(x_k W1_k)) = (H^T(dv*x_k)) W1_k,
    so we upload host-prescaled dv*x (bf16, replicated to every core) and gather
    x-rows directly -- no N-side x@W1 pass and no y AllGather.
    ef_k = de * (z_k W1_k + t b1_k) with z = H^T(dv x), t = H^T dv.
  * All four gather passes (edge-sorted x2, node-sorted x2) use
    dma_gather(prepare_only=True) + trigger_dma so the GpSimd engine only pays
    descriptor generation; transfers queue in the SWDGE ring and drain at SDMA
    rate, overlapped with the one-hot TensorE accumulation.
  * No dense H slabs: the second smooth (C_OUT padded 128) is pure gather too,
    reusing the same index/segment streams as the first smooth.
  * Degree scalings: dv folded into the uploaded x; de on the edge passes;
    dv^2 post-W2 (relu commutes with dv>=0); final dv on output;
    b2 via host-side rank-1 s1 = S@1 correction.
"""
import numpy as np
import ml_dtypes

BF16 = ml_dtypes.bfloat16
N, E, NNZ = 50000, 20000, 1600000
CONCAT, C_IN, C_HID = 4, 256, 256
C = CONCAT * C_HID            # 1024
C_OUT, C_OUT_P = 40, 128
W = 8
NPC_R, EPC_R = N // W, E // W           # 6250, 2500 real per core
NBLK, EBLK = 49, 20
NPC, EPC = NBLK * 128, EBLK * 128       # 6272, 2560 padded per core
NP_, EP_ = W * NPC, W * EPC             # 50176, 20480
NHALF = NP_ // 2                        # 25088 (int16 gather index split)
BATCH = 8                               # chunks per dma_gather (1024 idx max)
USE_PREP = False                        # prepare_only + trigger_dma pipelining
NQ = 4                                  # SWDGE queues used for gathers (ring
                                        # per queue -> overlapped transfers)


def _wrap_idx(idx):
    """[L] int -> [128, L/16] int16 wrapped layout, replicated across q7 cores."""
    L = len(idx)
    assert L % 16 == 0
    a = np.full((16, L // 16), 0, np.int16)
    a[np.arange(L) % 16, np.arange(L) // 16] = idx.astype(np.int16)
    return np.tile(a, (8, 1))


def _streams_var(rows, segpos, kbs):
    """Flat index stream + seg table with per-block chunk counts kbs."""
    total = sum(kbs)
    idx = np.zeros(total * 128, np.int64)
    seg = np.full((128, total), -1.0, np.float32)
    off = 0
    for r, p, kb in zip(rows, segpos, kbs):
        n = len(r)
        assert n <= kb * 128
        idx[off * 128:off * 128 + n] = r
        cols = off + np.arange(n) // 128
        seg[np.arange(n) % 128, cols] = p.astype(np.float32)
        off += kb
    return idx, seg.astype(BF16)


def _prep(node_idx, edge_idx, dv_is, de_inv, t_full):
    """Host-side prep: sorted gather streams + per-core scale tables."""
    nrow = (node_idx // NPC_R) * NPC + node_idx % NPC_R    # node -> padded row
    erow = (edge_idx // EPC_R) * EPC + edge_idx % EPC_R    # edge -> padded row
    p1 = np.argsort(edge_idx, kind="stable")
    e1, n1 = edge_idx[p1], nrow[p1]
    p2 = np.argsort(node_idx, kind="stable")
    n2, e2 = node_idx[p2], erow[p2]
    per = []
    for c in range(W):
        m1 = (e1 >= c * EPC_R) & (e1 < (c + 1) * EPC_R)
        el = e1[m1] - c * EPC_R
        nr = n1[m1]
        lo_rows, lo_pos, hi_rows, hi_pos = [], [], [], []
        for b in range(EBLK):
            mb = (el >= b * 128) & (el < (b + 1) * 128)
            rb, pb = nr[mb], el[mb] - b * 128
            lo = rb < NHALF
            lo_rows.append(rb[lo]); lo_pos.append(pb[lo])
            hi_rows.append(rb[~lo] - NHALF); hi_pos.append(pb[~lo])
        m2 = (n2 >= c * NPC_R) & (n2 < (c + 1) * NPC_R)
        nl = n2[m2] - c * NPC_R
        er = e2[m2]
        c_rows, c_pos = [], []
        for b in range(NBLK):
            mb = (nl >= b * 128) & (nl < (b + 1) * 128)
            c_rows.append(er[mb]); c_pos.append(nl[mb] - b * 128)
        per.append((lo_rows, lo_pos, hi_rows, hi_pos, c_rows, c_pos))
    kbA = [max(1, max((len(p[0][b]) + 127) // 128 for p in per)) for b in range(EBLK)]
    kbB = [max(1, max((len(p[2][b]) + 127) // 128 for p in per)) for b in range(EBLK)]
    kbC = [max(1, max((len(p[4][b]) + 127) // 128 for p in per)) for b in range(NBLK)]
    cores = []
    for c in range(W):
        lo_rows, lo_pos, hi_rows, hi_pos, c_rows, c_pos = per[c]
        iA, sA = _streams_var(lo_rows, lo_pos, kbA)
        iB, sB = _streams_var(hi_rows, hi_pos, kbB)
        iC, sC = _streams_var(c_rows, c_pos, kbC)
        dv = np.zeros(NPC, np.float32)
        dv[:NPC_R] = dv_is[c * NPC_R:(c + 1) * NPC_R]
        de = np.zeros(EPC, np.float32)
        de[:EPC_R] = de_inv[c * EPC_R:(c + 1) * EPC_R]
        t = np.zeros(EPC, np.float32)
        t[:EPC_R] = t_full[c * EPC_R:(c + 1) * EPC_R]
        cores.append(dict(
            idxA=_wrap_idx(iA), segA=sA, idxB=_wrap_idx(iB), segB=sB,
            idxC=_wrap_idx(iC), segC=sC,
            t_row=t.reshape(1, EPC).astype(BF16),
            dv_blk=dv.reshape(NBLK, 128).T.copy(),
            dvsq_blk=(dv * dv).reshape(NBLK, 128).T.copy(),
            de_blk=de.reshape(EBLK, 128).T.copy()))
    return cores, tuple(kbA), tuple(kbB), tuple(kbC)


def _build(kbA, kbB, kbC):
    import concourse.bass as bass
    import concourse.mybir as mybir
    from concourse import bacc, masks
    from concourse.tile import TileContext

    f32, bf16, i16 = mybir.dt.float32, mybir.dt.bfloat16, mybir.dt.int16
    sumA, sumB, sumC = sum(kbA), sum(kbB), sum(kbC)
    oA = np.concatenate([[0], np.cumsum(kbA)]).tolist()
    oB = np.concatenate([[0], np.cumsum(kbB)]).tolist()
    oC = np.concatenate([[0], np.cumsum(kbC)]).tolist()

    nc = bacc.Bacc("TRN2", num_devices=W, num_swdge_queues=NQ)
    T = lambda n, s, d=bf16: nc.dram_tensor(n, s, d, kind="ExternalInput")
    dvx = T("dvx", [NP_, C])                 # host-prescaled dv*x, branch-major
    W1 = T("W1", [CONCAT, C_IN, C_HID])
    b1c = T("b1c", [1, C])
    t_row_d = T("t_row", [1, EPC])
    W2p = T("W2p", [C, C_OUT_P])
    dv_blk = T("dv_blk", [128, NBLK], f32); dvsq_blk = T("dvsq_blk", [128, NBLK], f32)
    de_blk = T("de_blk", [128, EBLK], f32)
    idxA = T("idxA", [128, sumA * 8], i16); segA = T("segA", [128, sumA])
    idxB = T("idxB", [128, sumB * 8], i16); segB = T("segB", [128, sumB])
    idxC = T("idxC", [128, sumC * 8], i16); segC = T("segC", [128, sumC])
    iota_d = T("iota", [128, 128])
    out_own = nc.dram_tensor("out_own", [NPC, C_OUT_P], f32, kind="ExternalOutput")
    I = lambda n, s: nc.dram_tensor(n, s, bf16, kind="Internal")
    S = lambda n, s: nc.dram_tensor(n, s, bf16, kind="Internal", addr_space="Shared")
    ef_own, ef_full = I("ef_own", [EPC, C]), S("ef_full", [EP_, C])
    y2_own, y2_full = I("y2_own", [NPC, C_OUT_P]), S("y2_full", [NP_, C_OUT_P])
    ef2_own, ef2_full = I("ef2_own", [EPC, C_OUT_P]), S("ef2_full", [EP_, C_OUT_P])
    RG = [list(range(W))]
    AG = lambda i, o: nc.gpsimd.collective_compute(
        "AllGather", mybir.AluOpType.bypass, replica_groups=RG, ins=[i[:]], outs=[o[:]])

    with TileContext(nc) as tc:
        with tc.tile_pool(name="const", bufs=1) as cp:
            w1_sb = cp.tile([128, CONCAT * 2 * C_HID], bf16)     # f=(k*2+q) -> 256 cols
            for k in range(CONCAT):
                for q in range(2):
                    nc.sync.dma_start(
                        w1_sb[:, (k * 2 + q) * C_HID:(k * 2 + q + 1) * C_HID],
                        W1[k, q * 128:(q + 1) * 128, :])
            w2_sb = cp.tile([128, 8 * C_OUT_P], bf16)
            for f in range(8):
                nc.sync.dma_start(w2_sb[:, f * C_OUT_P:(f + 1) * C_OUT_P],
                                  W2p[f * 128:(f + 1) * 128, :])
            b1_sb = cp.tile([1, C], bf16); nc.sync.dma_start(b1_sb[:], b1c[:])
            t_sb = cp.tile([1, EPC], bf16); nc.sync.dma_start(t_sb[:], t_row_d[:])
            iota_sb = cp.tile([128, 128], bf16); nc.sync.dma_start(iota_sb[:], iota_d[:])
            ident = cp.tile([128, 128], bf16); masks.make_identity(nc, ident[:])
            dv_sb = cp.tile([128, NBLK], f32); nc.sync.dma_start(dv_sb[:], dv_blk[:])
            dvsq_sb = cp.tile([128, NBLK], f32); nc.sync.dma_start(dvsq_sb[:], dvsq_blk[:])
            de_sb = cp.tile([128, EBLK], f32); nc.sync.dma_start(de_sb[:], de_blk[:])
            iA = cp.tile([128, sumA * 8], i16); nc.sync.dma_start(iA[:], idxA[:])
            iB = cp.tile([128, sumB * 8], i16); nc.sync.dma_start(iB[:], idxB[:])
            iC = cp.tile([128, sumC * 8], i16); nc.scalar.dma_start(iC[:], idxC[:])
            sA = cp.tile([128, sumA], bf16); nc.scalar.dma_start(sA[:], segA[:])
            sB = cp.tile([128, sumB], bf16); nc.scalar.dma_start(sB[:], segB[:])
            sC = cp.tile([128, sumC], bf16); nc.scalar.dma_start(sC[:], segC[:])

            mm = lambda *a, **kw: nc.tensor.matmul(*a, skip_group_check=True, **kw)
            qsems = [nc.alloc_semaphore(f"gq{q}") for q in range(NQ)] if USE_PREP else None
            qctr = [0]
            qthr = [0] * NQ
            if USE_PREP:
                for q in range(NQ):
                    nc.gpsimd.sem_clear(qsems[q])

            def seg_pass(kb, off, idx_sb, seg_sb, src_ap, elem, pool, ps,
                         start_stream, stop_stream, tag, batch=BATCH):
                """Gather + one-hot-matmul accumulation for one block's stream."""
                for s in range(0, kb, batch):
                    nch = min(batch, kb - s)
                    k0 = off + s
                    g = pool.tile([128, batch, elem], bf16, tag=tag + "g")
                    gate = None
                    if USE_PREP:
                        q = qctr[0] % NQ
                        qctr[0] += 1
                        nc.gpsimd.dma_gather(
                            out_ap=g[:, :nch, :], in_ap=src_ap,
                            idxs_ap=idx_sb[:, k0 * 8:(k0 + nch) * 8],
                            num_idxs=nch * 128, num_idxs_reg=nch * 128,
                            elem_size=elem, prepare_only=True, sem=qsems[q],
                            queue_num=q)
                        nc.gpsimd.trigger_dma(count=None, queue_num=q)
                        qthr[q] += 16
                        gate = (qsems[q], qthr[q])
                    else:
                        q = qctr[0] % NQ
                        qctr[0] += 1
                        nc.gpsimd.dma_gather(
                            out_ap=g[:, :nch, :], in_ap=src_ap,
                            idxs_ap=idx_sb[:, k0 * 8:(k0 + nch) * 8],
                            num_idxs=nch * 128, num_idxs_reg=nch * 128,
                            elem_size=elem, queue_num=q)
                    oh = pool.tile([128, batch, 128], bf16, tag=tag + "o")
                    nc.vector.tensor_tensor(
                        out=oh[:, :nch, :],
                        in0=iota_sb[:, None, :].broadcast_to([128, nch, 128]),
                        in1=seg_sb[:, k0:k0 + nch, None].broadcast_to([128, nch, 128]),
                        op=mybir.AluOpType.is_equal)
                    if gate is not None:
                        nc.tensor.wait_ge(gate[0], gate[1])
                    for j in range(nch):
                        first = start_stream and (s == 0 and j == 0)
                        last = stop_stream and (s + j == kb - 1)
                        for h in range((elem + 511) // 512):
                            w_ = min(512, elem - h * 512)
                            mm(ps[:, h * 512:h * 512 + w_],
                               lhsT=oh[:, j, :], rhs=g[:, j, h * 512:h * 512 + w_],
                               start=first, stop=last)

            # ---- phase B': z = H^T(dv x); ef = de * (z_k W1_k + t b1_k) ----
            with tc.tile_pool(name="pb", bufs=5) as pb, \
                 tc.tile_pool(name="pbz", bufs=2, space="PSUM") as pbz, \
                 tc.tile_pool(name="pbt", bufs=1, space="PSUM") as pbt, \
                 tc.tile_pool(name="pbe", bufs=1, space="PSUM") as pbe:
                for b in range(EBLK):
                    pz = pbz.tile([128, C], f32, tag="pz")
                    seg_pass(kbA[b], oA[b], iA, sA, dvx[0:NHALF, :], C,
                             pb, pz, True, False, "A")
                    seg_pass(kbB[b], oB[b], iB, sB, dvx[NHALF:NP_, :], C,
                             pb, pz, False, True, "A")
                    z_sb = pb.tile([128, C], bf16, tag="zsb")
                    nc.vector.tensor_copy(z_sb[:], pz[:])
                    pt = pbt.tile([128, C], bf16, tag="pt")
                    for f in range(8):
                        nc.tensor.transpose(pt[:, f * 128:(f + 1) * 128],
                                            z_sb[:, f * 128:(f + 1) * 128], ident[:])
                    zt_sb = pb.tile([128, C], bf16, tag="ztsb")
                    nc.vector.tensor_copy(zt_sb[:], pt[:])
                    pef = pbe.tile([128, C], f32, tag="pef")
                    tb = t_sb[:, b * 128:(b + 1) * 128]
                    mm(pef[:, :512], lhsT=tb, rhs=b1_sb[:, :512], start=True, stop=False)
                    mm(pef[:, 512:], lhsT=tb, rhs=b1_sb[:, 512:], start=True, stop=False)
                    for k in range(CONCAT):
                        for qh in range(2):
                            f = 2 * k + qh
                            mm(pef[:, k * C_HID:(k + 1) * C_HID],
                               lhsT=zt_sb[:, f * 128:(f + 1) * 128],
                               rhs=w1_sb[:, f * C_HID:(f + 1) * C_HID],
                               start=False, stop=(qh == 1))
                    ef_sb = pb.tile([128, C], bf16, tag="efsb")
                    nc.vector.tensor_tensor(
                        out=ef_sb[:], in0=pef[:],
                        in1=de_sb[:, b:b + 1].broadcast_to([128, C]),
                        op=mybir.AluOpType.mult)
                    nc.sync.dma_start(ef_own[b * 128:(b + 1) * 128, :], ef_sb[:])
            AG(ef_own, ef_full)

            # ---- phase C: u = relu(H ef); y2 = dv^2 * (u @ W2) ----
            with tc.tile_pool(name="pc", bufs=5) as pc, \
                 tc.tile_pool(name="pcz", bufs=3, space="PSUM") as pcz, \
                 tc.tile_pool(name="pct", bufs=1, space="PSUM") as pct, \
                 tc.tile_pool(name="pcy", bufs=1, space="PSUM") as pcy:
                for b in range(NBLK):
                    pu = pcz.tile([128, C], f32, tag="pu")
                    seg_pass(kbC[b], oC[b], iC, sC, ef_full[:], C, pc, pu,
                             True, True, "C")
                    u_sb = pc.tile([128, C], bf16, tag="usb")
                    nc.scalar.activation(out=u_sb[:], in_=pu[:],
                                         func=mybir.ActivationFunctionType.Relu)
                    pt = pct.tile([128, C], bf16, tag="ptc")
                    for f in range(8):
                        nc.tensor.transpose(pt[:, f * 128:(f + 1) * 128],
                                            u_sb[:, f * 128:(f + 1) * 128], ident[:])
                    ut_sb = pc.tile([128, C], bf16, tag="utsb")
                    nc.vector.tensor_copy(ut_sb[:], pt[:])
                    py2 = pcy.tile([128, C_OUT_P], f32, tag="py2")
                    for f in range(8):
                        mm(py2[:], lhsT=ut_sb[:, f * 128:(f + 1) * 128],
                           rhs=w2_sb[:, f * C_OUT_P:(f + 1) * C_OUT_P],
                           start=(f == 0), stop=(f == 7))
                    y2_sb = pc.tile([128, C_OUT_P], bf16, tag="y2sb")
                    nc.vector.tensor_tensor(
                        out=y2_sb[:], in0=py2[:],
                        in1=dvsq_sb[:, b:b + 1].broadcast_to([128, C_OUT_P]),
                        op=mybir.AluOpType.mult)
                    nc.sync.dma_start(y2_own[b * 128:(b + 1) * 128, :], y2_sb[:])
            AG(y2_own, y2_full)

            # ---- phase D: ef2 = de * (H^T y2), same streams as B' ----
            with tc.tile_pool(name="pd", bufs=6) as pd, \
                 tc.tile_pool(name="pdp", bufs=4, space="PSUM") as pdp:
                for b in range(EBLK):
                    ps2 = pdp.tile([128, C_OUT_P], f32, tag="ps2")
                    seg_pass(kbA[b], oA[b], iA, sA, y2_full[0:NHALF, :],
                             C_OUT_P, pd, ps2, True, False, "D")
                    seg_pass(kbB[b], oB[b], iB, sB, y2_full[NHALF:NP_, :],
                             C_OUT_P, pd, ps2, False, True, "D")
                    e2_sb = pd.tile([128, C_OUT_P], bf16, tag="e2sb")
                    nc.vector.tensor_tensor(
                        out=e2_sb[:], in0=ps2[:],
                        in1=de_sb[:, b:b + 1].broadcast_to([128, C_OUT_P]),
                        op=mybir.AluOpType.mult)
                    nc.sync.dma_start(ef2_own[b * 128:(b + 1) * 128, :], e2_sb[:])
            AG(ef2_own, ef2_full)

            # ---- phase E: res = dv * (H ef2), same stream as C ----
            with tc.tile_pool(name="pe", bufs=6) as pe_, \
                 tc.tile_pool(name="pep", bufs=4, space="PSUM") as pep:
                for b in range(NBLK):
                    po = pep.tile([128, C_OUT_P], f32, tag="po")
                    seg_pass(kbC[b], oC[b], iC, sC, ef2_full[:], C_OUT_P,
                             pe_, po, True, True, "F")
                    o_sb = pe_.tile([128, C_OUT_P], f32, tag="osb")
                    nc.vector.tensor_tensor(
                        out=o_sb[:], in0=po[:],
                        in1=dv_sb[:, b:b + 1].broadcast_to([128, C_OUT_P]),
                        op=mybir.AluOpType.mult)
                    nc.sync.dma_start(out_own[b * 128:(b + 1) * 128, :], o_sb[:])
    nc.finalize()
    return nc


_CACHE = {}


def kernel(x_list, W1, b1, W2, b2, node_idx, edge_idx, n_edges, _trace=False,
           _tmpdir=None):
    from concourse import bass_utils
    x_list = np.asarray(x_list, np.float32); W1 = np.asarray(W1, np.float32)
    b1 = np.asarray(b1, np.float32); W2 = np.asarray(W2, np.float32)
    b2 = np.asarray(b2, np.float32)
    node_idx = np.asarray(node_idx, np.int32); edge_idx = np.asarray(edge_idx, np.int32)

    dv = np.bincount(node_idx, minlength=N).astype(np.float32)
    de = np.bincount(edge_idx, minlength=E).astype(np.float32)
    dv_is = np.where(dv > 0, 1.0 / np.sqrt(np.maximum(dv, 1.0)), 0.0).astype(np.float32)
    de_inv = np.where(de > 0, 1.0 / np.maximum(de, 1.0), 0.0).astype(np.float32)
    # t = H^T dv (edge sums of dv) for the b1 rank-1 term
    t_full = np.bincount(edge_idx, weights=dv_is[node_idx], minlength=E).astype(np.float32)
    # s1 = S @ 1 for the host-side b2 rank-1 term
    ef_t = t_full * de_inv
    s1 = dv_is * np.bincount(node_idx, weights=ef_t[edge_idx], minlength=N)

    cores, kbA, kbB, kbC = _prep(node_idx, edge_idx, dv_is, de_inv, t_full)
    key = (kbA, kbB, kbC)
    if key not in _CACHE:
        _CACHE[key] = _build(kbA, kbB, kbC)
    nc = _CACHE[key]

    W2p = np.zeros((C, C_OUT_P), np.float32)
    W2p[:, :C_OUT] = W2
    iota_np = np.tile(np.arange(128, dtype=np.float32), (128, 1))
    # dvx[padded row, (k,cin)] = dv[n] * x[k, n, cin], branch-major columns
    dvx_real = (x_list.transpose(1, 0, 2).reshape(N, C)
                * dv_is[:, None]).astype(BF16)
    dvx_p = np.zeros((NP_, C), BF16)
    for c in range(W):
        dvx_p[c * NPC:c * NPC + NPC_R] = dvx_real[c * NPC_R:(c + 1) * NPC_R]
    in_maps = []
    for c in range(W):
        m = dict(dvx=dvx_p, W1=W1.astype(BF16),
                 b1c=b1.reshape(1, C).astype(BF16), W2p=W2p.astype(BF16),
                 iota=iota_np.astype(BF16), **cores[c])
        in_maps.append(m)
    try:
        res = bass_utils.run_bass_kernel_spmd(nc, in_maps, core_ids=list(range(W)),
                                              trace=_trace, tmpdir=_tmpdir)
    except ModuleNotFoundError:
        res = bass_utils.run_bass_kernel_spmd(nc, in_maps, core_ids=list(range(W)),
                                              trace=False)
    out = np.empty((N, C_OUT), np.float32)
    for c in range(W):
        out[c * NPC_R:(c + 1) * NPC_R] = res.results[c]["out_own"][:NPC_R, :C_OUT]
    out += np.outer(s1, b2)
    kernel._last = res
    return out
